# revision 15
# baseline (speedup 1.0000x reference)
"""Trainium2 Bass kernel for a 2-layer Mamba stack (BasicLayer). v2.

Per layer: LayerNorm -> in_proj (1024->4096) -> causal depthwise conv(k=4)
+ SiLU -> x_proj (2048->96) -> dt_proj + softplus -> selective scan over
L=2048 -> gate with SiLU(z) -> out_proj (2048->1024).

Sharding: tensor-parallel over d_inner (2048 / 8 cores = 256 channels per
core).  Cross-core sums (x_proj and out_proj contractions) are AllReduced
on-chip in bf16, out_proj split in token halves so the collectives overlap
compute.  All matmuls run in bf16 (fp32 matmul is 4 cycles/row vs 1 for
bf16).  Transposes use the DMA xbar (dma_start_transpose) instead of the
PE+PSUM path.  The depthwise conv and the D*u skip term are expressed as
diagonal-matrix matmuls on the PE so the vector engine only carries the
scan itself plus the B/C elementwise products.  softplus(x) is computed as
Ln(1+Exp(x)) -- both functions live in the same activation table, and the
LayerNorm rstd is the only per-chunk table swap (Sqrt).
"""

import numpy as np

try:
    import concourse.bass as bass
except ImportError:  # pragma: no cover
    import sys

    sys.path.insert(0, "/opt/trn_rl_repo")
    import concourse.bass as bass

import concourse.bacc as bacc
import concourse.mybir as mybir
import concourse.tile as tile
from concourse.bass_utils import run_bass_kernel_spmd

F32 = mybir.dt.float32
BF16 = mybir.dt.bfloat16
AF = mybir.ActivationFunctionType
ALU = mybir.AluOpType

B, L = 2, 2048
DM, DI, DS, DTR, DCONV, DEPTH = 1024, 2048, 16, 64, 4, 2
EPS = 1e-5
NCORES = 8
DL = DI // NCORES          # 256 channels per core
NDT = DL // 128            # 2 channel tiles per core
T = B * L                  # 4096 tokens
NCH = T // 512             # 8 chunks of 512 tokens
LH = L // 2                # token half for out AllReduce chunking

# knobs
YT_POOL_N = set()          # scan ns whose yt-mul runs on gpsimd instead of DVE
BT_POOL_N = set()


def build_nc(apply_norm_w: bool, apply_norm_b: bool, fake_cc: bool = False):
    nc = bacc.Bacc(
        "TRN2",
        target_bir_lowering=False,
        debug=False,
        enable_asserts=False,
        num_devices=NCORES,
    )

    # ---- I/O declarations (per-core data supplied via in_maps) ----
    x_dram = nc.dram_tensor("x_tm", [T, DM], BF16, kind="ExternalInput")
    w_inT = nc.dram_tensor("w_inT", [2, DEPTH, DM, 4 * 128], BF16, kind="ExternalInput")
    w_outT = nc.dram_tensor("w_outT", [2, DEPTH, DL, DM], BF16, kind="ExternalInput")
    w_xpT = nc.dram_tensor("w_xpT", [2, DEPTH, DL, 96], BF16, kind="ExternalInput")
    w_dtT = nc.dram_tensor("w_dtT", [2, DEPTH, DTR, DL], BF16, kind="ExternalInput")
    conv_dg = nc.dram_tensor("conv_dg", [2, DEPTH, NDT, DCONV, 128, 128], BF16,
                             kind="ExternalInput")
    d_dg = nc.dram_tensor("d_dg", [2, DEPTH, NDT, 128, 128], BF16, kind="ExternalInput")
    conv_b = nc.dram_tensor("conv_b_c", [DEPTH, DL, 1], F32, kind="ExternalInput")
    dt_b = nc.dram_tensor("dt_b_c", [DEPTH, DL, 1], F32, kind="ExternalInput")
    a_log = nc.dram_tensor("a_log_c", [DEPTH, DL, DS], F32, kind="ExternalInput")
    ident = nc.dram_tensor("ident_bf", [128, 128], BF16, kind="ExternalInput")
    if apply_norm_w:
        nwb = nc.dram_tensor("norm_w_bc", [DEPTH, 128, DM], F32, kind="ExternalInput")
    if apply_norm_b:
        nbb = nc.dram_tensor("norm_b_bc", [DEPTH, 128, DM], F32, kind="ExternalInput")
    out_dram = nc.dram_tensor("out_tm", [T, DM], F32, kind="ExternalOutput")

    groups = [list(range(NCORES))]

    def all_reduce(src_ap, dst_ap):
        if fake_cc:
            nc.sync.dma_start(dst_ap, src_ap)
        else:
            nc.gpsimd.collective_compute(
                "AllReduce", ALU.add, replica_groups=groups,
                ins=[src_ap], outs=[dst_ap],
            )

    with tile.TileContext(nc, num_cores=NCORES) as tc:
        with (
            tc.tile_pool(name="wp", bufs=1) as wp,
            tc.tile_pool(name="lnp", bufs=2) as lnp,
            tc.tile_pool(name="sp", bufs=2) as sp,
            tc.tile_pool(name="dp", bufs=2) as dp,
            tc.tile_pool(name="bbp", bufs=3) as bbp,
            tc.tile_pool(name="bcp", bufs=3) as bcp,
            tc.tile_pool(name="psA", bufs=2, space="PSUM") as psA,
            tc.tile_pool(name="psY", bufs=1, space="PSUM") as psY,
            tc.tile_pool(name="psS", bufs=2, space="PSUM") as psS,
            tc.tile_pool(name="dram", bufs=2, space="DRAM") as dram,
        ):
            ident_sb = wp.tile([128, 128], BF16, tag="ident")
            nc.sync.dma_start(ident_sb[:], ident[:, :])
            eps_sb = wp.tile([128, 1], F32, tag="eps")
            nc.vector.memset(eps_sb[:], EPS)
            one_sb = wp.tile([128, 1], F32, tag="one")
            nc.vector.memset(one_sb[:], 1.0)

            # hsrc(b, row0) -> AP of 128 input rows for this layer
            hsrc_l0 = [x_dram.ap()[0:L, :], x_dram.ap()[L:T, :]]

            def hsrc_l0_get(b, row0):
                return hsrc_l0[b][row0:row0 + 128, :]

            hget = hsrc_l0_get

            for l in range(DEPTH):
                # ---- per-layer weights ----
                winT, woutT, wxpT, wdtT = [], [], [], []
                for hl in range(2):
                    row = []
                    for kt in range(8):
                        w = wp.tile([128, 512], BF16, tag=f"winT{hl}_{kt}",
                                    name=f"winT{l}_{hl}_{kt}")
                        nc.sync.dma_start(w[:], w_inT[hl, l, kt * 128:(kt + 1) * 128, :])
                        row.append(w)
                    winT.append(row)
                    row = []
                    for j in range(NDT):
                        w = wp.tile([128, DM], BF16, tag=f"woutT{hl}_{j}",
                                    name=f"woutT{l}_{hl}_{j}")
                        nc.sync.dma_start(w[:], w_outT[hl, l, j * 128:(j + 1) * 128, :])
                        row.append(w)
                    woutT.append(row)
                    row = []
                    for j in range(NDT):
                        w = wp.tile([128, 96], BF16, tag=f"wxpT{hl}_{j}",
                                    name=f"wxpT{l}_{hl}_{j}")
                        nc.sync.dma_start(w[:], w_xpT[hl, l, j * 128:(j + 1) * 128, :])
                        row.append(w)
                    wxpT.append(row)
                    w = wp.tile([DTR, DL], BF16, tag=f"wdtT{hl}", name=f"wdtT{l}_{hl}")
                    nc.sync.dma_start(w[:], w_dtT[hl, l, :, :])
                    wdtT.append(w)
                cdg, ddg, convb, dtb, Asb = [], [], [], [], []
                for hl in range(2):
                    crow = []
                    drow = []
                    for j in range(NDT):
                        krow = []
                        for k in range(DCONV):
                            cw = wp.tile([128, 128], BF16, tag=f"cdg{hl}_{j}_{k}",
                                         name=f"cdg{l}_{hl}_{j}_{k}")
                            nc.sync.dma_start(cw[:], conv_dg[hl, l, j, k, :, :])
                            krow.append(cw)
                        crow.append(krow)
                        dd = wp.tile([128, 128], BF16, tag=f"ddg{hl}_{j}",
                                     name=f"ddg{l}_{hl}_{j}")
                        nc.sync.dma_start(dd[:], d_dg[hl, l, j, :, :])
                        drow.append(dd)
                    cdg.append(crow)
                    ddg.append(drow)
                for j in range(NDT):
                    cb = wp.tile([128, 1], F32, tag=f"convb{j}")
                    nc.sync.dma_start(cb[:], conv_b[l, j * 128:(j + 1) * 128, :])
                    convb.append(cb)
                    db = wp.tile([128, 1], F32, tag=f"dtb{j}")
                    nc.sync.dma_start(db[:], dt_b[l, j * 128:(j + 1) * 128, :])
                    dtb.append(db)
                    at = wp.tile([128, DS], F32, tag=f"alog{j}")
                    nc.sync.dma_start(at[:], a_log[l, j * 128:(j + 1) * 128, :])
                    ae = wp.tile([128, DS], F32, tag=f"aexp{j}")
                    nc.scalar.activation(ae[:], at[:], AF.Exp)
                    an = wp.tile([128, DS], F32, tag=f"aneg{j}")
                    nc.vector.tensor_scalar_mul(an[:], ae[:], -1.0)
                    Asb.append(an)
                if apply_norm_w:
                    nw_sb = wp.tile([128, DM], F32, tag="nwsb")
                    nc.sync.dma_start(nw_sb[:], nwb[l, :, :])
                if apply_norm_b:
                    nb_sb = wp.tile([128, DM], F32, tag="nbsb")
                    nc.sync.dma_start(nb_sb[:], nbb[l, :, :])

                # per-layer resident SBUF tensors
                u_sb = [wp.tile([128, T], BF16, tag=f"u{j}", name=f"u{l}_{j}")
                        for j in range(NDT)]
                z_sb = [wp.tile([128, T], BF16, tag=f"z{j}", name=f"z{l}_{j}")
                        for j in range(NDT)]

                # DRAM staging for collectives
                xdbl_in = [dram.tile([96, L], BF16, tag=f"xdbli{b}",
                                     name=f"xdbli{l}_{b}") for b in range(B)]
                xdbl_sh = [dram.tile([96, L], BF16, tag=f"xdblo{b}", addr_space="Shared",
                                     name=f"xdblo{l}_{b}") for b in range(B)]
                odt = F32 if l == DEPTH - 1 else BF16
                out_part = [[dram.tile([LH, DM], odt, tag=f"opart{l}_{b}_{h}",
                                       name=f"opart{l}_{b}_{h}") for h in range(2)]
                            for b in range(B)]
                hred = [[dram.tile([LH, DM], odt, tag=f"hred{l}_{b}_{h}", addr_space="Shared",
                                   name=f"hred{l}_{b}_{h}") for h in range(2)]
                        for b in range(B)]

                # ================= phase A: LN + transpose + in_proj + conv ===========
                prev_ue = [None, None]
                for ci in range(NCH):
                    b = ci // 4
                    tok0 = ci * 512
                    xa_t, nbias_t = [], []
                    nvar4 = lnp.tile([128, 4], F32, tag="nvar4", bufs=2)
                    # pass 1: stats (Identity+Square accumulate; any act table)
                    for tti in range(4):
                        row0 = (ci % 4) * 512 + tti * 128
                        xa = lnp.tile([128, DM], BF16, tag="xa", bufs=4)
                        nc.sync.dma_start(xa[:], hget(b, row0))
                        xa_t.append(xa)
                        scr = lnp.tile([128, DM], BF16, tag="scr", bufs=1)
                        sums = lnp.tile([128, 1], F32, tag="sums", bufs=4)
                        nc.scalar.activation(scr[:], xa[:], AF.Identity, accum_out=sums[:])
                        sumsq = lnp.tile([128, 1], F32, tag="sumsq", bufs=4)
                        nc.scalar.activation(scr[:], xa[:], AF.Square, accum_out=sumsq[:])
                        mean = lnp.tile([128, 1], F32, tag="mean", bufs=4)
                        nc.vector.tensor_scalar_mul(mean[:], sums[:], 1.0 / DM)
                        msq = lnp.tile([128, 1], F32, tag="msq", bufs=4)
                        nc.vector.tensor_scalar_mul(msq[:], sumsq[:], 1.0 / DM)
                        nc.vector.scalar_tensor_tensor(
                            nvar4[:, tti:tti + 1], mean[:], mean[:], msq[:],
                            ALU.mult, ALU.subtract
                        )
                        nbias_t.append(mean)
                    # pass 2: rstd (one Sqrt per chunk -- single table-swap cluster)
                    std4 = lnp.tile([128, 4], F32, tag="std4", bufs=2)
                    nc.scalar.activation(std4[:], nvar4[:], AF.Sqrt,
                                         bias=eps_sb[:], scale=-1.0)
                    rstd4 = lnp.tile([128, 4], F32, tag="rstd4", bufs=2)
                    nc.vector.reciprocal(rstd4[:], std4[:])
                    hn_pack = lnp.tile([128, 4096], BF16, tag="hnpack", bufs=1)
                    for tti in range(4):
                        mean = nbias_t[tti]
                        rstd = rstd4[:, tti:tti + 1]
                        nbias = lnp.tile([128, 1], F32, tag="nbias", bufs=4)
                        nc.vector.scalar_tensor_tensor(
                            nbias[:], mean[:], -1.0, rstd, ALU.mult, ALU.mult
                        )
                        hcol = hn_pack[:, tti * DM:(tti + 1) * DM]
                        if apply_norm_w or apply_norm_b:
                            hn0 = lnp.tile([128, DM], F32, tag="hn0", bufs=2)
                            nc.vector.tensor_scalar(
                                hn0[:], xa_t[tti][:], rstd, nbias[:],
                                ALU.mult, ALU.add,
                            )
                            if apply_norm_w and apply_norm_b:
                                hn1 = lnp.tile([128, DM], F32, tag="hn1", bufs=2)
                                nc.vector.tensor_mul(hn1[:], hn0[:], nw_sb[:])
                                nc.vector.tensor_add(hcol, hn1[:], nb_sb[:])
                            elif apply_norm_w:
                                nc.vector.tensor_mul(hcol, hn0[:], nw_sb[:])
                            else:
                                nc.vector.tensor_add(hcol, hn0[:], nb_sb[:])
                        else:
                            nc.vector.tensor_scalar(
                                hcol, xa_t[tti][:], rstd, nbias[:],
                                ALU.mult, ALU.add,
                            )
                    # transpose via DMA xbar: hnT[p, kt, t] = hn_pack[t', kt*128+p]
                    hnT = lnp.tile([128, 8, 512], BF16, tag="hnT", bufs=1)
                    for tti in range(4):
                        nc.scalar.dma_start_transpose(
                            hnT[:, :, tti * 128:(tti + 1) * 128],
                            hn_pack[:, tti * DM:(tti + 1) * DM],
                        )
                    # in_proj + conv + silu + x_proj
                    for mt in range(4):
                        pm = psA.tile([128, 512], F32, tag="pm")
                        for hl in range(2):
                            for kt in range(8):
                                nc.tensor.matmul(
                                    pm[:],
                                    winT[hl][kt][:, mt * 128:(mt + 1) * 128],
                                    hnT[:, kt, :],
                                    start=(hl == 0 and kt == 0),
                                    stop=(hl == 1 and kt == 7),
                                )
                        if mt < NDT:
                            j = mt
                            ue = sp.tile([128, 515], BF16, tag=f"ue{j}", bufs=2)
                            if ci % 4 == 0:
                                nc.vector.memset(ue[:, 0:3], 0.0)
                            else:
                                nc.vector.tensor_copy(
                                    ue[:, 0:3], prev_ue[j][:, 512:515]
                                )
                            nc.scalar.copy(ue[:, 3:515], pm[:])
                            prev_ue[j] = ue
                            pcv = psA.tile([128, 512], F32, tag="pm")
                            for hl in range(2):
                                for k in range(DCONV):
                                    nc.tensor.matmul(
                                        pcv[:], cdg[hl][j][k][:], ue[:, k:k + 512],
                                        start=(hl == 0 and k == 0),
                                        stop=(hl == 1 and k == DCONV - 1),
                                    )
                            nc.scalar.activation(
                                u_sb[j][:, tok0:tok0 + 512], pcv[:],
                                AF.Silu, bias=convb[j][:],
                            )
                        else:
                            j = mt - NDT
                            nc.scalar.activation(
                                z_sb[j][:, tok0:tok0 + 512], pm[:], AF.Silu
                            )
                    px = psA.tile([96, 512], F32, tag="pm")
                    for hl in range(2):
                        for j in range(NDT):
                            nc.tensor.matmul(
                                px[:], wxpT[hl][j][:], u_sb[j][:, tok0:tok0 + 512],
                                start=(hl == 0 and j == 0),
                                stop=(hl == 1 and j == NDT - 1),
                            )
                    xdc = sp.tile([96, 512], BF16, tag="xdc", bufs=2)
                    nc.scalar.copy(xdc[:], px[:])
                    ctok = (ci % 4) * 512
                    nc.sync.dma_start(xdbl_in[b][:, ctok:ctok + 512], xdc[:])

                    if ci % 4 == 3:
                        all_reduce(xdbl_in[b].opt(), xdbl_sh[b].opt())

                # ============= phase D: dt + scan; phase E: out_proj =============
                for b in range(B):
                    xrd = sp.tile([DTR, L], BF16, tag="xrd")
                    nc.sync.dma_start(xrd[:], xdbl_sh[b][0:DTR, :])
                    dts, dus = [], []
                    for j in range(NDT):
                        dt_j = dp.tile([128, L], BF16, tag=f"dt{j}", bufs=1,
                                       name=f"dt{l}_{b}_{j}")
                        evs = []
                        for q in range(4):
                            pdm = psS.tile([128, 512], F32, tag="ps")
                            for hl in range(2):
                                nc.tensor.matmul(
                                    pdm[:],
                                    wdtT[hl][:, j * 128:(j + 1) * 128],
                                    xrd[:, q * 512:(q + 1) * 512],
                                    start=(hl == 0), stop=(hl == 1),
                                )
                            ev = sp.tile([128, 512], F32, tag="ev", bufs=4)
                            nc.scalar.activation(ev[:], pdm[:], AF.Exp, bias=dtb[j][:])
                            evs.append(ev)
                        for q in range(4):
                            nc.scalar.activation(
                                dt_j[:, q * 512:(q + 1) * 512], evs[q][:],
                                AF.Ln, bias=one_sb[:],
                            )
                        du_j = dp.tile([128, L], BF16, tag=f"du{j}", bufs=1,
                                       name=f"du{l}_{b}_{j}")
                        nc.vector.tensor_mul(
                            du_j[:], dt_j[:], u_sb[j][:, b * L:(b + 1) * L]
                        )
                        dts.append(dt_j)
                        dus.append(du_j)
                    for j in range(NDT):
                        y_ps = psY.tile([128, L], F32, tag="yps")
                        for n in range(DS):
                            pb = bbp.tile([128, L], BF16, tag="pb")
                            nc.sync.dma_start(
                                pb[:],
                                xdbl_sh[b][DTR + n:DTR + n + 1, :].to_broadcast((128, L)),
                            )
                            pc = bcp.tile([128, L], BF16, tag="pc")
                            nc.sync.dma_start(
                                pc[:],
                                xdbl_sh[b][DTR + DS + n:DTR + DS + n + 1, :]
                                .to_broadcast((128, L)),
                            )
                            ada = dp.tile([128, L], BF16, tag="ada")
                            nc.scalar.activation(
                                ada[:], dts[j][:], AF.Exp, scale=Asb[j][:, n:n + 1]
                            )
                            bt = dp.tile([128, L], BF16, tag="bt")
                            if n in BT_POOL_N:
                                nc.gpsimd.tensor_tensor(bt[:], dus[j][:], pb[:], ALU.mult)
                            else:
                                nc.vector.tensor_mul(bt[:], dus[j][:], pb[:])
                            hs = dp.tile([128, L], BF16, tag="hs")
                            nc.vector.tensor_tensor_scan(
                                hs[:], ada[:], bt[:], 0.0, ALU.mult, ALU.add
                            )
                            if n in YT_POOL_N:
                                nc.gpsimd.tensor_tensor(hs[:], hs[:], pc[:], ALU.mult)
                            else:
                                nc.vector.tensor_mul(hs[:], hs[:], pc[:])
                            for q in range(4):
                                nc.tensor.matmul(
                                    y_ps[:, q * 512:(q + 1) * 512],
                                    ident_sb[:],
                                    hs[:, q * 512:(q + 1) * 512],
                                    start=(n == 0), stop=False,
                                )
                        # D*u skip term closes the accumulation
                        for hl in range(2):
                            for q in range(4):
                                nc.tensor.matmul(
                                    y_ps[:, q * 512:(q + 1) * 512],
                                    ddg[hl][j][:],
                                    u_sb[j][:, b * L + q * 512: b * L + (q + 1) * 512],
                                    start=False, stop=(hl == 1),
                                )
                        y2 = dp.tile([128, L], BF16, tag=f"y2{j}", bufs=1)
                        nc.vector.tensor_mul(
                            y2[:], y_ps[:], z_sb[j][:, b * L:(b + 1) * L]
                        )
                        dts[j] = None
                        if j == 0:
                            y2s = [y2]
                        else:
                            y2s.append(y2)
                    # phase E: out_proj in token halves, AllReduce each half
                    for h in range(2):
                        for tt in range(8):
                            t0 = h * LH + tt * 128
                            for nt in range(2):
                                po = psS.tile([128, 512], F32, tag="ps")
                                for hl in range(2):
                                    for j in range(NDT):
                                        nc.tensor.matmul(
                                            po[:],
                                            y2s[j][:, t0:t0 + 128],
                                            woutT[hl][j][:, nt * 512:(nt + 1) * 512],
                                            start=(hl == 0 and j == 0),
                                            stop=(hl == 1 and j == NDT - 1),
                                        )
                                oc = sp.tile([128, 512], odt, tag="oc", bufs=3)
                                nc.scalar.copy(oc[:], po[:])
                                nc.sync.dma_start(
                                    out_part[b][h][tt * 128:(tt + 1) * 128,
                                                   nt * 512:(nt + 1) * 512],
                                    oc[:],
                                )
                        all_reduce(out_part[b][h].opt(), hred[b][h].opt())

                def mk_hget(hred_l):
                    def _g(b, row0):
                        h = row0 // LH
                        r = row0 % LH
                        return hred_l[b][h][r:r + 128, :]
                    return _g

                hget = mk_hget(hred)

            # final: last layer hred is f32 -- straight DRAM->DRAM copy
            for b in range(B):
                for h in range(2):
                    nc.sync.dma_start(
                        out_dram[b * L + h * LH: b * L + (h + 1) * LH, :],
                        hred[b][h][:, :],
                    )

    nc.compile()
    return nc


_CACHE = {}


def _get_nc(apply_norm_w, apply_norm_b, fake_cc=False):
    key = (apply_norm_w, apply_norm_b, fake_cc)
    if key not in _CACHE:
        _CACHE[key] = build_nc(apply_norm_w, apply_norm_b, fake_cc)
    return _CACHE[key]


def make_in_maps(x, norm_w, norm_b, in_proj_w, conv_w, conv_b, x_proj_w,
                 dt_proj_w, dt_proj_b, A_log, D, out_proj_w,
                 apply_norm_w, apply_norm_b):
    bf = mybir.dt.np(BF16)
    f = lambda a: np.ascontiguousarray(np.asarray(a), dtype=np.float32)
    fb = lambda a: np.ascontiguousarray(np.asarray(a, dtype=np.float32).astype(bf))

    def hilo(a):
        a = np.asarray(a, dtype=np.float32)
        hi = a.astype(bf)
        lo = (a - hi.astype(np.float32)).astype(bf)
        return np.ascontiguousarray(np.stack([hi, lo], axis=0))

    x_tm = fb(np.asarray(x).reshape(T, DM))
    in_proj_w = np.asarray(in_proj_w)
    conv_w = np.asarray(conv_w)
    D_np = np.asarray(D)
    in_maps = []
    for c in range(NCORES):
        sl = slice(c * DL, (c + 1) * DL)
        w_in_rows = np.concatenate(
            [in_proj_w[:, sl, :], in_proj_w[:, DI + c * DL: DI + (c + 1) * DL, :]],
            axis=1,
        )  # (DEPTH, 512, 1024)
        cdg = np.zeros((DEPTH, NDT, DCONV, 128, 128), dtype=np.float32)
        ddg = np.zeros((DEPTH, NDT, 128, 128), dtype=np.float32)
        for li in range(DEPTH):
            for j in range(NDT):
                ch = slice(c * DL + j * 128, c * DL + (j + 1) * 128)
                for k in range(DCONV):
                    np.fill_diagonal(cdg[li, j, k], conv_w[li, ch, 0, k])
                np.fill_diagonal(ddg[li, j], D_np[li, ch])
        m = {
            "x_tm": x_tm,
            "w_inT": hilo(w_in_rows.transpose(0, 2, 1)),
            "w_outT": hilo(np.asarray(out_proj_w)[:, :, sl].transpose(0, 2, 1)),
            "w_xpT": hilo(np.asarray(x_proj_w)[:, :, sl].transpose(0, 2, 1)),
            "w_dtT": hilo(np.asarray(dt_proj_w)[:, sl, :].transpose(0, 2, 1)),
            "conv_dg": hilo(cdg),
            "d_dg": hilo(ddg),
            "conv_b_c": f(np.asarray(conv_b)[:, sl][..., None]),
            "dt_b_c": f(np.asarray(dt_proj_b)[:, sl][..., None]),
            "a_log_c": f(np.asarray(A_log)[:, sl, :]),
            "ident_bf": np.eye(128, dtype=np.float32).astype(bf),
        }
        if apply_norm_w:
            m["norm_w_bc"] = f(np.broadcast_to(np.asarray(norm_w)[:, None, :], (DEPTH, 128, DM)))
        if apply_norm_b:
            m["norm_b_bc"] = f(np.broadcast_to(np.asarray(norm_b)[:, None, :], (DEPTH, 128, DM)))
        in_maps.append(m)
    return in_maps


def kernel(x, x_size, norm_w, norm_b, in_proj_w, conv_w, conv_b, x_proj_w,
           dt_proj_w, dt_proj_b, A_log, D, out_proj_w, **_unused):
    apply_norm_w = not np.allclose(np.asarray(norm_w), 1.0)
    apply_norm_b = not np.allclose(np.asarray(norm_b), 0.0)
    nc = _get_nc(apply_norm_w, apply_norm_b)
    in_maps = make_in_maps(
        x, norm_w, norm_b, in_proj_w, conv_w, conv_b, x_proj_w,
        dt_proj_w, dt_proj_b, A_log, D, out_proj_w,
        apply_norm_w, apply_norm_b,
    )
    res = run_bass_kernel_spmd(nc, in_maps, core_ids=list(range(NCORES)))
    return res.results[0]["out_tm"].reshape(B, L, DM).astype(np.float32)


# revision 19
# speedup vs baseline: 1.1765x; 1.1765x over previous
"""Trainium2 Bass kernel for a 2-layer Mamba stack (BasicLayer). v2.

Per layer: LayerNorm -> in_proj (1024->4096) -> causal depthwise conv(k=4)
+ SiLU -> x_proj (2048->96) -> dt_proj + softplus -> selective scan over
L=2048 -> gate with SiLU(z) -> out_proj (2048->1024).

Sharding: tensor-parallel over d_inner (2048 / 8 cores = 256 channels per
core).  Cross-core sums (x_proj and out_proj contractions) are AllReduced
on-chip in bf16, out_proj split in token halves so the collectives overlap
compute.  All matmuls run in bf16 (fp32 matmul is 4 cycles/row vs 1 for
bf16).  Transposes use the DMA xbar (dma_start_transpose) instead of the
PE+PSUM path.  The depthwise conv and the D*u skip term are expressed as
diagonal-matrix matmuls on the PE so the vector engine only carries the
scan itself plus the B/C elementwise products.  softplus(x) is computed as
Ln(1+Exp(x)) -- both functions live in the same activation table, and the
LayerNorm rstd is the only per-chunk table swap (Sqrt).
"""

import numpy as np

try:
    import concourse.bass as bass
except ImportError:  # pragma: no cover
    import sys

    sys.path.insert(0, "/opt/trn_rl_repo")
    import concourse.bass as bass

import concourse.bacc as bacc
import concourse.mybir as mybir
import concourse.tile as tile
from concourse.bass_utils import run_bass_kernel_spmd

F32 = mybir.dt.float32
BF16 = mybir.dt.bfloat16
AF = mybir.ActivationFunctionType
ALU = mybir.AluOpType

B, L = 2, 2048
DM, DI, DS, DTR, DCONV, DEPTH = 1024, 2048, 16, 64, 4, 2
EPS = 1e-5
NCORES = 8
DL = DI // NCORES          # 256 channels per core
NDT = DL // 128            # 2 channel tiles per core
T = B * L                  # 4096 tokens
NCH = T // 512             # 8 chunks of 512 tokens
LH = L // 2                # token half for out AllReduce chunking

# knobs
YT_POOL_N = set()          # scan ns whose yt-mul runs on gpsimd instead of DVE
BT_POOL_N = set()


def build_nc(apply_norm_w: bool, apply_norm_b: bool, fake_cc: bool = False):
    nc = bacc.Bacc(
        "TRN2",
        target_bir_lowering=False,
        debug=False,
        enable_asserts=False,
        num_devices=NCORES,
    )

    # ---- I/O declarations (per-core data supplied via in_maps) ----
    x_dram = nc.dram_tensor("x_tm", [T, DM], BF16, kind="ExternalInput")
    # packed per-layer weights: big contiguous blocks to minimize DMA count
    w_inT = nc.dram_tensor("w_inT", [DEPTH, 128, 2 * 8 * 512], BF16, kind="ExternalInput")
    w_outT = nc.dram_tensor("w_outT", [DEPTH, 128, 2 * NDT * DM], BF16, kind="ExternalInput")
    w_xpT = nc.dram_tensor("w_xpT", [DEPTH, 128, 2 * NDT * 96], BF16, kind="ExternalInput")
    w_dtT = nc.dram_tensor("w_dtT", [DEPTH, DTR, 2 * DL], BF16, kind="ExternalInput")
    conv_dg = nc.dram_tensor("conv_dg", [DEPTH, 128, 2 * NDT * DCONV * 128], BF16,
                             kind="ExternalInput")
    d_dg = nc.dram_tensor("d_dg", [DEPTH, 128, 2 * NDT * 128], BF16, kind="ExternalInput")
    conv_b = nc.dram_tensor("conv_b_c", [DEPTH, DL, 1], F32, kind="ExternalInput")
    dt_b = nc.dram_tensor("dt_b_c", [DEPTH, DL, 1], F32, kind="ExternalInput")
    a_log = nc.dram_tensor("a_log_c", [DEPTH, DL, DS], F32, kind="ExternalInput")
    ident = nc.dram_tensor("ident_bf", [128, 128], BF16, kind="ExternalInput")
    if apply_norm_w:
        nwb = nc.dram_tensor("norm_w_bc", [DEPTH, 128, DM], F32, kind="ExternalInput")
    if apply_norm_b:
        nbb = nc.dram_tensor("norm_b_bc", [DEPTH, 128, DM], F32, kind="ExternalInput")
    out_dram = nc.dram_tensor("out_tm", [T, DM], F32, kind="ExternalOutput")

    groups = [list(range(NCORES))]

    def all_reduce(src_ap, dst_ap):
        if fake_cc:
            nc.sync.dma_start(dst_ap, src_ap)
        else:
            nc.gpsimd.collective_compute(
                "AllReduce", ALU.add, replica_groups=groups,
                ins=[src_ap], outs=[dst_ap],
            )

    with tile.TileContext(nc, num_cores=NCORES) as tc:
        with (
            tc.tile_pool(name="wp", bufs=1) as wp,
            tc.tile_pool(name="lnp", bufs=2) as lnp,
            tc.tile_pool(name="sp", bufs=2) as sp,
            tc.tile_pool(name="dp", bufs=2) as dp,
            tc.tile_pool(name="bbp", bufs=3) as bbp,
            tc.tile_pool(name="bcp", bufs=3) as bcp,
            tc.tile_pool(name="psA", bufs=2, space="PSUM") as psA,
            tc.tile_pool(name="psY", bufs=1, space="PSUM") as psY,
            tc.tile_pool(name="psS", bufs=2, space="PSUM") as psS,
            tc.tile_pool(name="dram", bufs=2, space="DRAM") as dram,
        ):
            ident_sb = wp.tile([128, 128], BF16, tag="ident")
            nc.sync.dma_start(ident_sb[:], ident[:, :])
            eps_sb = wp.tile([128, 1], F32, tag="eps")
            nc.vector.memset(eps_sb[:], EPS)
            one_sb = wp.tile([128, 1], F32, tag="one")
            nc.vector.memset(one_sb[:], 1.0)

            # hsrc(b, row0) -> AP of 128 input rows for this layer
            hsrc_l0 = [x_dram.ap()[0:L, :], x_dram.ap()[L:T, :]]

            def hsrc_l0_get(b, row0):
                return hsrc_l0[b][row0:row0 + 128, :]

            hget = hsrc_l0_get

            for l in range(DEPTH):
                # ---- per-layer weights ----
                win_all = wp.tile([128, 2 * 8 * 512], BF16, tag="winall")
                nc.sync.dma_start(win_all[:], w_inT[l, :, :])
                wout_all = wp.tile([128, 2 * NDT * DM], BF16, tag="woutall")
                nc.sync.dma_start(wout_all[:], w_outT[l, :, :])
                wxp_all = wp.tile([128, 2 * NDT * 96], BF16, tag="wxpall")
                nc.sync.dma_start(wxp_all[:], w_xpT[l, :, :])
                wdt_all = wp.tile([DTR, 2 * DL], BF16, tag="wdtall")
                nc.sync.dma_start(wdt_all[:], w_dtT[l, :, :])
                cd_all = wp.tile([128, 2 * NDT * DCONV * 128], BF16, tag="cdall")
                nc.sync.dma_start(cd_all[:], conv_dg[l, :, :])
                dd_all = wp.tile([128, 2 * NDT * 128], BF16, tag="ddall")
                nc.sync.dma_start(dd_all[:], d_dg[l, :, :])
                winT = [[win_all[:, (hl * 8 + kt) * 512:(hl * 8 + kt + 1) * 512]
                         for kt in range(8)] for hl in range(2)]
                woutT = [[wout_all[:, (hl * NDT + j) * DM:(hl * NDT + j + 1) * DM]
                          for j in range(NDT)] for hl in range(2)]
                wxpT = [[wxp_all[:, (hl * NDT + j) * 96:(hl * NDT + j + 1) * 96]
                         for j in range(NDT)] for hl in range(2)]
                wdtT = [wdt_all[:, hl * DL:(hl + 1) * DL] for hl in range(2)]
                cdg = [[[cd_all[:, ((hl * NDT + j) * DCONV + k) * 128:
                                ((hl * NDT + j) * DCONV + k + 1) * 128]
                         for k in range(DCONV)] for j in range(NDT)] for hl in range(2)]
                ddg = [[dd_all[:, (hl * NDT + j) * 128:(hl * NDT + j + 1) * 128]
                        for j in range(NDT)] for hl in range(2)]
                convb, dtb, Asb = [], [], []
                for j in range(NDT):
                    cb = wp.tile([128, 1], F32, tag=f"convb{j}")
                    nc.sync.dma_start(cb[:], conv_b[l, j * 128:(j + 1) * 128, :])
                    convb.append(cb)
                    db = wp.tile([128, 1], F32, tag=f"dtb{j}")
                    nc.sync.dma_start(db[:], dt_b[l, j * 128:(j + 1) * 128, :])
                    dtb.append(db)
                    at = wp.tile([128, DS], F32, tag=f"alog{j}")
                    nc.sync.dma_start(at[:], a_log[l, j * 128:(j + 1) * 128, :])
                    ae = wp.tile([128, DS], F32, tag=f"aexp{j}")
                    nc.scalar.activation(ae[:], at[:], AF.Exp)
                    an = wp.tile([128, DS], F32, tag=f"aneg{j}")
                    nc.vector.tensor_scalar_mul(an[:], ae[:], -1.0)
                    Asb.append(an)
                if apply_norm_w:
                    nw_sb = wp.tile([128, DM], F32, tag="nwsb")
                    nc.sync.dma_start(nw_sb[:], nwb[l, :, :])
                if apply_norm_b:
                    nb_sb = wp.tile([128, DM], F32, tag="nbsb")
                    nc.sync.dma_start(nb_sb[:], nbb[l, :, :])

                # per-layer resident SBUF tensors
                u_sb = [wp.tile([128, T], BF16, tag=f"u{j}", name=f"u{l}_{j}")
                        for j in range(NDT)]
                z_sb = [wp.tile([128, T], BF16, tag=f"z{j}", name=f"z{l}_{j}")
                        for j in range(NDT)]

                # DRAM staging for collectives
                xdbl_in = [dram.tile([96, L], BF16, tag=f"xdbli{b}",
                                     name=f"xdbli{l}_{b}") for b in range(B)]
                xdbl_sh = [dram.tile([96, L], BF16, tag=f"xdblo{b}", addr_space="Shared",
                                     name=f"xdblo{l}_{b}") for b in range(B)]
                odt = F32 if l == DEPTH - 1 else BF16
                out_part = [[dram.tile([LH, DM], odt, tag=f"opart{l}_{b}_{h}",
                                       name=f"opart{l}_{b}_{h}") for h in range(2)]
                            for b in range(B)]
                hred = [[dram.tile([LH, DM], odt, tag=f"hred{l}_{b}_{h}",
                                   addr_space="Shared",
                                   name=f"hred{l}_{b}_{h}") for h in range(2)]
                        for b in range(B)]
                hred_ap = [[hred[b][h].opt() for h in range(2)] for b in range(B)]
                if l == DEPTH - 1:
                    final_hred = hred

                # ================= phase A: LN + transpose + in_proj + conv ===========
                prev_ue = [None, None]
                for ci in range(NCH):
                    b = ci // 4
                    tok0 = ci * 512
                    xa_t = [], 
                    xa_t = []
                    var4 = lnp.tile([128, 4], F32, tag="var4", bufs=2)
                    bna8 = lnp.tile([128, 8], F32, tag="bna8", bufs=2)
                    # stats on DVE: bn_stats (2x 512) + bn_aggr per t-tile
                    for tti in range(4):
                        row0 = (ci % 4) * 512 + tti * 128
                        xa = lnp.tile([128, DM], BF16, tag="xa", bufs=4)
                        nc.sync.dma_start(xa[:], hget(b, row0))
                        xa_t.append(xa)
                        bst = lnp.tile([128, 12], F32, tag="bst", bufs=2)
                        nc.vector.bn_stats(bst[:, 0:6], xa[:, 0:512])
                        nc.vector.bn_stats(bst[:, 6:12], xa[:, 512:1024])
                        nc.vector.bn_aggr(bna8[:, 2 * tti:2 * tti + 2], bst[:])
                        nc.vector.tensor_copy(var4[:, tti:tti + 1],
                                              bna8[:, 2 * tti + 1:2 * tti + 2])
                    # rstd: one Sqrt per chunk -- single table-swap cluster
                    std4 = lnp.tile([128, 4], F32, tag="std4", bufs=2)
                    nc.scalar.activation(std4[:], var4[:], AF.Sqrt, bias=eps_sb[:])
                    rstd4 = lnp.tile([128, 4], F32, tag="rstd4", bufs=2)
                    nc.vector.reciprocal(rstd4[:], std4[:])
                    hn_pack = lnp.tile([128, 4096], BF16, tag="hnpack", bufs=1)
                    for tti in range(4):
                        mean = bna8[:, 2 * tti:2 * tti + 1]
                        rstd = rstd4[:, tti:tti + 1]
                        nbias = lnp.tile([128, 1], F32, tag="nbias", bufs=4)
                        nc.vector.scalar_tensor_tensor(
                            nbias[:], mean, -1.0, rstd, ALU.mult, ALU.mult
                        )
                        hcol = hn_pack[:, tti * DM:(tti + 1) * DM]
                        if apply_norm_w or apply_norm_b:
                            hn0 = lnp.tile([128, DM], F32, tag="hn0", bufs=2)
                            nc.vector.tensor_scalar(
                                hn0[:], xa_t[tti][:], rstd, nbias[:],
                                ALU.mult, ALU.add,
                            )
                            if apply_norm_w and apply_norm_b:
                                hn1 = lnp.tile([128, DM], F32, tag="hn1", bufs=2)
                                nc.vector.tensor_mul(hn1[:], hn0[:], nw_sb[:])
                                nc.vector.tensor_add(hcol, hn1[:], nb_sb[:])
                            elif apply_norm_w:
                                nc.vector.tensor_mul(hcol, hn0[:], nw_sb[:])
                            else:
                                nc.vector.tensor_add(hcol, hn0[:], nb_sb[:])
                        else:
                            nc.vector.tensor_scalar(
                                hcol, xa_t[tti][:], rstd, nbias[:],
                                ALU.mult, ALU.add,
                            )
                    # transpose via DMA xbar: hnT[p, kt, t] = hn_pack[t', kt*128+p]
                    hnT = lnp.tile([128, 8, 512], BF16, tag="hnT", bufs=1)
                    for tti in range(4):
                        nc.sync.dma_start_transpose(
                            hnT[:, :, tti * 128:(tti + 1) * 128],
                            hn_pack[:, tti * DM:(tti + 1) * DM],
                        )
                    # in_proj + conv + silu + x_proj
                    for mt in range(4):
                        pm = psA.tile([128, 512], F32, tag="pm")
                        for hl in range(2):
                            for kt in range(8):
                                nc.tensor.matmul(
                                    pm[:],
                                    winT[hl][kt][:, mt * 128:(mt + 1) * 128],
                                    hnT[:, kt, :],
                                    start=(hl == 0 and kt == 0),
                                    stop=(hl == 1 and kt == 7),
                                )
                        if mt < NDT:
                            j = mt
                            ue = sp.tile([128, 515], BF16, tag=f"ue{j}", bufs=2)
                            if ci % 4 == 0:
                                nc.vector.memset(ue[:, 0:3], 0.0)
                            else:
                                nc.vector.tensor_copy(
                                    ue[:, 0:3], prev_ue[j][:, 512:515]
                                )
                            nc.scalar.copy(ue[:, 3:515], pm[:])
                            prev_ue[j] = ue
                            pcv = psA.tile([128, 512], F32, tag="pm")
                            for hl in range(2):
                                for k in range(DCONV):
                                    nc.tensor.matmul(
                                        pcv[:], cdg[hl][j][k][:], ue[:, k:k + 512],
                                        start=(hl == 0 and k == 0),
                                        stop=(hl == 1 and k == DCONV - 1),
                                    )
                            nc.scalar.activation(
                                u_sb[j][:, tok0:tok0 + 512], pcv[:],
                                AF.Silu, bias=convb[j][:],
                            )
                        else:
                            j = mt - NDT
                            nc.scalar.activation(
                                z_sb[j][:, tok0:tok0 + 512], pm[:], AF.Silu
                            )
                    px = psA.tile([96, 512], F32, tag="pm")
                    for hl in range(2):
                        for j in range(NDT):
                            nc.tensor.matmul(
                                px[:], wxpT[hl][j][:], u_sb[j][:, tok0:tok0 + 512],
                                start=(hl == 0 and j == 0),
                                stop=(hl == 1 and j == NDT - 1),
                            )
                    xdc = sp.tile([96, 512], BF16, tag="xdc", bufs=2)
                    nc.scalar.copy(xdc[:], px[:])
                    ctok = (ci % 4) * 512
                    nc.sync.dma_start(xdbl_in[b][:, ctok:ctok + 512], xdc[:])

                    if ci % 4 == 3:
                        all_reduce(xdbl_in[b].opt(), xdbl_sh[b].opt())

                # ============= phase D: dt + scan; phase E: out_proj =============
                for b in range(B):
                    xrd = sp.tile([DTR, L], BF16, tag="xrd")
                    nc.sync.dma_start(xrd[:], xdbl_sh[b][0:DTR, :])
                    dts, dus = [], []
                    for j in range(NDT):
                        dt_j = dp.tile([128, L], BF16, tag=f"dt{j}", bufs=1,
                                       name=f"dt{l}_{b}_{j}")
                        evs = []
                        for q in range(4):
                            pdm = psS.tile([128, 512], F32, tag="ps")
                            for hl in range(2):
                                nc.tensor.matmul(
                                    pdm[:],
                                    wdtT[hl][:, j * 128:(j + 1) * 128],
                                    xrd[:, q * 512:(q + 1) * 512],
                                    start=(hl == 0), stop=(hl == 1),
                                )
                            ev = sp.tile([128, 512], F32, tag="ev", bufs=4)
                            nc.scalar.activation(ev[:], pdm[:], AF.Exp, bias=dtb[j][:])
                            evs.append(ev)
                        for q in range(4):
                            nc.scalar.activation(
                                dt_j[:, q * 512:(q + 1) * 512], evs[q][:],
                                AF.Ln, bias=one_sb[:],
                            )
                        du_j = dp.tile([128, L], BF16, tag=f"du{j}", bufs=1,
                                       name=f"du{l}_{b}_{j}")
                        nc.vector.tensor_mul(
                            du_j[:], dt_j[:], u_sb[j][:, b * L:(b + 1) * L]
                        )
                        dts.append(dt_j)
                        dus.append(du_j)
                    for j in range(NDT):
                        y_ps = psY.tile([128, L], F32, tag="yps")
                        for n in range(DS):
                            pb = bbp.tile([128, L], BF16, tag="pb")
                            nc.sync.dma_start(
                                pb[:],
                                xdbl_sh[b][DTR + n:DTR + n + 1, :].to_broadcast((128, L)),
                            )
                            pc = bcp.tile([128, L], BF16, tag="pc")
                            nc.sync.dma_start(
                                pc[:],
                                xdbl_sh[b][DTR + DS + n:DTR + DS + n + 1, :]
                                .to_broadcast((128, L)),
                            )
                            ada = dp.tile([128, L], BF16, tag="ada")
                            nc.scalar.activation(
                                ada[:], dts[j][:], AF.Exp, scale=Asb[j][:, n:n + 1]
                            )
                            bt = dp.tile([128, L], BF16, tag="bt")
                            if n in BT_POOL_N:
                                nc.gpsimd.tensor_tensor(bt[:], dus[j][:], pb[:], ALU.mult)
                            else:
                                nc.vector.tensor_mul(bt[:], dus[j][:], pb[:])
                            hs = dp.tile([128, L], BF16, tag="hs")
                            nc.vector.tensor_tensor_scan(
                                hs[:], ada[:], bt[:], 0.0, ALU.mult, ALU.add
                            )
                            if n in YT_POOL_N:
                                nc.gpsimd.tensor_tensor(hs[:], hs[:], pc[:], ALU.mult)
                            else:
                                nc.vector.tensor_mul(hs[:], hs[:], pc[:])
                            for q in range(4):
                                nc.tensor.matmul(
                                    y_ps[:, q * 512:(q + 1) * 512],
                                    ident_sb[:],
                                    hs[:, q * 512:(q + 1) * 512],
                                    start=(n == 0), stop=False,
                                )
                        # D*u skip term closes the accumulation
                        for hl in range(2):
                            for q in range(4):
                                nc.tensor.matmul(
                                    y_ps[:, q * 512:(q + 1) * 512],
                                    ddg[hl][j][:],
                                    u_sb[j][:, b * L + q * 512: b * L + (q + 1) * 512],
                                    start=False, stop=(hl == 1),
                                )
                        y2 = dp.tile([128, L], BF16, tag=f"y2{j}", bufs=1)
                        nc.vector.tensor_mul(
                            y2[:], y_ps[:], z_sb[j][:, b * L:(b + 1) * L]
                        )
                        dts[j] = None
                        if j == 0:
                            y2s = [y2]
                        else:
                            y2s.append(y2)
                    # phase E: out_proj in token halves, AllReduce each half
                    for h in range(2):
                        for tt in range(8):
                            t0 = h * LH + tt * 128
                            for nt in range(2):
                                po = psS.tile([128, 512], F32, tag="ps")
                                for hl in range(2):
                                    for j in range(NDT):
                                        nc.tensor.matmul(
                                            po[:],
                                            y2s[j][:, t0:t0 + 128],
                                            woutT[hl][j][:, nt * 512:(nt + 1) * 512],
                                            start=(hl == 0 and j == 0),
                                            stop=(hl == 1 and j == NDT - 1),
                                        )
                                oc = sp.tile([128, 512], odt, tag="oc", bufs=3)
                                if nt == 0:
                                    nc.scalar.copy(oc[:], po[:])
                                else:
                                    nc.vector.tensor_copy(oc[:], po[:])
                                nc.sync.dma_start(
                                    out_part[b][h][tt * 128:(tt + 1) * 128,
                                                   nt * 512:(nt + 1) * 512],
                                    oc[:],
                                )
                        all_reduce(out_part[b][h].opt(), hred_ap[b][h])

                def mk_hget(hred_l):
                    def _g(b, row0):
                        h = row0 // LH
                        r = row0 % LH
                        return hred_l[b][h][r:r + 128, :]
                    return _g

                hget = mk_hget(hred)

            # final: straight DRAM->DRAM copy of the f32 last-layer result
            for b in range(B):
                for h in range(2):
                    nc.sync.dma_start(
                        out_dram[b * L + h * LH: b * L + (h + 1) * LH, :],
                        final_hred[b][h][:, :],
                    )

    nc.compile()
    return nc


_CACHE = {}


def _get_nc(apply_norm_w, apply_norm_b, fake_cc=False):
    key = (apply_norm_w, apply_norm_b, fake_cc)
    if key not in _CACHE:
        _CACHE[key] = build_nc(apply_norm_w, apply_norm_b, fake_cc)
    return _CACHE[key]


def make_in_maps(x, norm_w, norm_b, in_proj_w, conv_w, conv_b, x_proj_w,
                 dt_proj_w, dt_proj_b, A_log, D, out_proj_w,
                 apply_norm_w, apply_norm_b):
    bf = mybir.dt.np(BF16)
    f = lambda a: np.ascontiguousarray(np.asarray(a), dtype=np.float32)
    fb = lambda a: np.ascontiguousarray(np.asarray(a, dtype=np.float32).astype(bf))

    def hilo(a):
        a = np.asarray(a, dtype=np.float32)
        hi = a.astype(bf)
        lo = (a - hi.astype(np.float32)).astype(bf)
        return np.ascontiguousarray(np.stack([hi, lo], axis=0))

    def pack_in(a):  # [2, DEPTH, DM, 512] -> [DEPTH, 128, 2*8*512]
        a = a.reshape(2, DEPTH, 8, 128, 512)
        return np.ascontiguousarray(
            a.transpose(1, 3, 0, 2, 4).reshape(DEPTH, 128, 2 * 8 * 512))

    def pack_pj(a, w):  # [2, DEPTH, DL, w] -> [DEPTH, 128, 2*NDT*w]
        a = a.reshape(2, DEPTH, NDT, 128, w)
        return np.ascontiguousarray(
            a.transpose(1, 3, 0, 2, 4).reshape(DEPTH, 128, 2 * NDT * w))

    def pack_dt(a):  # [2, DEPTH, DTR, DL] -> [DEPTH, DTR, 2*DL]
        return np.ascontiguousarray(
            a.transpose(1, 2, 0, 3).reshape(DEPTH, DTR, 2 * DL))

    def pack_cd(a):  # [2, DEPTH, NDT, DCONV, 128, 128] -> [DEPTH, 128, 2*NDT*DCONV*128]
        return np.ascontiguousarray(
            a.transpose(1, 4, 0, 2, 3, 5).reshape(DEPTH, 128, 2 * NDT * DCONV * 128))

    def pack_dd(a):  # [2, DEPTH, NDT, 128, 128] -> [DEPTH, 128, 2*NDT*128]
        return np.ascontiguousarray(
            a.transpose(1, 3, 0, 2, 4).reshape(DEPTH, 128, 2 * NDT * 128))

    x_tm = fb(np.asarray(x).reshape(T, DM))
    in_proj_w = np.asarray(in_proj_w)
    conv_w = np.asarray(conv_w)
    D_np = np.asarray(D)
    in_maps = []
    for c in range(NCORES):
        sl = slice(c * DL, (c + 1) * DL)
        w_in_rows = np.concatenate(
            [in_proj_w[:, sl, :], in_proj_w[:, DI + c * DL: DI + (c + 1) * DL, :]],
            axis=1,
        )  # (DEPTH, 512, 1024)
        cdg = np.zeros((DEPTH, NDT, DCONV, 128, 128), dtype=np.float32)
        ddg = np.zeros((DEPTH, NDT, 128, 128), dtype=np.float32)
        for li in range(DEPTH):
            for j in range(NDT):
                ch = slice(c * DL + j * 128, c * DL + (j + 1) * 128)
                for k in range(DCONV):
                    np.fill_diagonal(cdg[li, j, k], conv_w[li, ch, 0, k])
                np.fill_diagonal(ddg[li, j], D_np[li, ch])
        m = {
            "x_tm": x_tm,
            "w_inT": pack_in(hilo(w_in_rows.transpose(0, 2, 1))),
            "w_outT": pack_pj(hilo(np.asarray(out_proj_w)[:, :, sl].transpose(0, 2, 1)), DM),
            "w_xpT": pack_pj(hilo(np.asarray(x_proj_w)[:, :, sl].transpose(0, 2, 1)), 96),
            "w_dtT": pack_dt(hilo(np.asarray(dt_proj_w)[:, sl, :].transpose(0, 2, 1))),
            "conv_dg": pack_cd(hilo(cdg)),
            "d_dg": pack_dd(hilo(ddg)),
            "conv_b_c": f(np.asarray(conv_b)[:, sl][..., None]),
            "dt_b_c": f(np.asarray(dt_proj_b)[:, sl][..., None]),
            "a_log_c": f(np.asarray(A_log)[:, sl, :]),
            "ident_bf": np.eye(128, dtype=np.float32).astype(bf),
        }
        if apply_norm_w:
            m["norm_w_bc"] = f(np.broadcast_to(np.asarray(norm_w)[:, None, :], (DEPTH, 128, DM)))
        if apply_norm_b:
            m["norm_b_bc"] = f(np.broadcast_to(np.asarray(norm_b)[:, None, :], (DEPTH, 128, DM)))
        in_maps.append(m)
    return in_maps


def kernel(x, x_size, norm_w, norm_b, in_proj_w, conv_w, conv_b, x_proj_w,
           dt_proj_w, dt_proj_b, A_log, D, out_proj_w, **_unused):
    apply_norm_w = not np.allclose(np.asarray(norm_w), 1.0)
    apply_norm_b = not np.allclose(np.asarray(norm_b), 0.0)
    nc = _get_nc(apply_norm_w, apply_norm_b)
    in_maps = make_in_maps(
        x, norm_w, norm_b, in_proj_w, conv_w, conv_b, x_proj_w,
        dt_proj_w, dt_proj_b, A_log, D, out_proj_w,
        apply_norm_w, apply_norm_b,
    )
    res = run_bass_kernel_spmd(nc, in_maps, core_ids=list(range(NCORES)))
    return res.results[0]["out_tm"].reshape(B, L, DM).astype(np.float32)


# revision 20
# speedup vs baseline: 1.1769x; 1.0003x over previous
"""Trainium2 Bass kernel for a 2-layer Mamba stack (BasicLayer). v2.

Per layer: LayerNorm -> in_proj (1024->4096) -> causal depthwise conv(k=4)
+ SiLU -> x_proj (2048->96) -> dt_proj + softplus -> selective scan over
L=2048 -> gate with SiLU(z) -> out_proj (2048->1024).

Sharding: tensor-parallel over d_inner (2048 / 8 cores = 256 channels per
core).  Cross-core sums (x_proj and out_proj contractions) are AllReduced
on-chip in bf16, out_proj split in token halves so the collectives overlap
compute.  All matmuls run in bf16 (fp32 matmul is 4 cycles/row vs 1 for
bf16).  Transposes use the DMA xbar (dma_start_transpose) instead of the
PE+PSUM path.  The depthwise conv and the D*u skip term are expressed as
diagonal-matrix matmuls on the PE so the vector engine only carries the
scan itself plus the B/C elementwise products.  softplus(x) is computed as
Ln(1+Exp(x)) -- both functions live in the same activation table, and the
LayerNorm rstd is the only per-chunk table swap (Sqrt).
"""

import numpy as np

try:
    import concourse.bass as bass
except ImportError:  # pragma: no cover
    import sys

    sys.path.insert(0, "/opt/trn_rl_repo")
    import concourse.bass as bass

import concourse.bacc as bacc
import concourse.mybir as mybir
import concourse.tile as tile
from concourse.bass_utils import run_bass_kernel_spmd

F32 = mybir.dt.float32
BF16 = mybir.dt.bfloat16
AF = mybir.ActivationFunctionType
ALU = mybir.AluOpType

B, L = 2, 2048
DM, DI, DS, DTR, DCONV, DEPTH = 1024, 2048, 16, 64, 4, 2
EPS = 1e-5
NCORES = 8
DL = DI // NCORES          # 256 channels per core
NDT = DL // 128            # 2 channel tiles per core
T = B * L                  # 4096 tokens
NCH = T // 512             # 8 chunks of 512 tokens
LH = L // 2                # token half for out AllReduce chunking

# knobs
YT_POOL_N = set()          # scan ns whose yt-mul runs on gpsimd instead of DVE
BT_POOL_N = {1, 3, 5, 7, 9, 11, 13, 15}


def build_nc(apply_norm_w: bool, apply_norm_b: bool, fake_cc: bool = False):
    nc = bacc.Bacc(
        "TRN2",
        target_bir_lowering=False,
        debug=False,
        enable_asserts=False,
        num_devices=NCORES,
    )

    # ---- I/O declarations (per-core data supplied via in_maps) ----
    x_dram = nc.dram_tensor("x_tm", [T, DM], BF16, kind="ExternalInput")
    # packed per-layer weights: big contiguous blocks to minimize DMA count
    w_inT = nc.dram_tensor("w_inT", [DEPTH, 128, 2 * 8 * 512], BF16, kind="ExternalInput")
    w_outT = nc.dram_tensor("w_outT", [DEPTH, 128, 2 * NDT * DM], BF16, kind="ExternalInput")
    w_xpT = nc.dram_tensor("w_xpT", [DEPTH, 128, 2 * NDT * 96], BF16, kind="ExternalInput")
    w_dtT = nc.dram_tensor("w_dtT", [DEPTH, DTR, 2 * DL], BF16, kind="ExternalInput")
    conv_dg = nc.dram_tensor("conv_dg", [DEPTH, 128, 2 * NDT * DCONV * 128], BF16,
                             kind="ExternalInput")
    d_dg = nc.dram_tensor("d_dg", [DEPTH, 128, 2 * NDT * 128], BF16, kind="ExternalInput")
    conv_b = nc.dram_tensor("conv_b_c", [DEPTH, DL, 1], F32, kind="ExternalInput")
    dt_b = nc.dram_tensor("dt_b_c", [DEPTH, DL, 1], F32, kind="ExternalInput")
    a_log = nc.dram_tensor("a_log_c", [DEPTH, DL, DS], F32, kind="ExternalInput")
    ident = nc.dram_tensor("ident_bf", [128, 128], BF16, kind="ExternalInput")
    if apply_norm_w:
        nwb = nc.dram_tensor("norm_w_bc", [DEPTH, 128, DM], F32, kind="ExternalInput")
    if apply_norm_b:
        nbb = nc.dram_tensor("norm_b_bc", [DEPTH, 128, DM], F32, kind="ExternalInput")
    out_dram = nc.dram_tensor("out_tm", [T, DM], F32, kind="ExternalOutput")

    groups = [list(range(NCORES))]

    def all_reduce(src_ap, dst_ap):
        if fake_cc:
            nc.sync.dma_start(dst_ap, src_ap)
        else:
            nc.gpsimd.collective_compute(
                "AllReduce", ALU.add, replica_groups=groups,
                ins=[src_ap], outs=[dst_ap],
            )

    with tile.TileContext(nc, num_cores=NCORES) as tc:
        with (
            tc.tile_pool(name="wp", bufs=1) as wp,
            tc.tile_pool(name="lnp", bufs=2) as lnp,
            tc.tile_pool(name="sp", bufs=2) as sp,
            tc.tile_pool(name="dp", bufs=2) as dp,
            tc.tile_pool(name="bbp", bufs=3) as bbp,
            tc.tile_pool(name="bcp", bufs=3) as bcp,
            tc.tile_pool(name="psA", bufs=2, space="PSUM") as psA,
            tc.tile_pool(name="psY", bufs=1, space="PSUM") as psY,
            tc.tile_pool(name="psS", bufs=2, space="PSUM") as psS,
            tc.tile_pool(name="dram", bufs=2, space="DRAM") as dram,
        ):
            ident_sb = wp.tile([128, 128], BF16, tag="ident")
            nc.sync.dma_start(ident_sb[:], ident[:, :])
            eps_sb = wp.tile([128, 1], F32, tag="eps")
            nc.vector.memset(eps_sb[:], EPS)
            one_sb = wp.tile([128, 1], F32, tag="one")
            nc.vector.memset(one_sb[:], 1.0)

            # hsrc(b, row0) -> AP of 128 input rows for this layer
            hsrc_l0 = [x_dram.ap()[0:L, :], x_dram.ap()[L:T, :]]

            def hsrc_l0_get(b, row0):
                return hsrc_l0[b][row0:row0 + 128, :]

            hget = hsrc_l0_get

            for l in range(DEPTH):
                # ---- per-layer weights ----
                win_all = wp.tile([128, 2 * 8 * 512], BF16, tag="winall")
                nc.sync.dma_start(win_all[:], w_inT[l, :, :])
                wout_all = wp.tile([128, 2 * NDT * DM], BF16, tag="woutall")
                nc.sync.dma_start(wout_all[:], w_outT[l, :, :])
                wxp_all = wp.tile([128, 2 * NDT * 96], BF16, tag="wxpall")
                nc.sync.dma_start(wxp_all[:], w_xpT[l, :, :])
                wdt_all = wp.tile([DTR, 2 * DL], BF16, tag="wdtall")
                nc.sync.dma_start(wdt_all[:], w_dtT[l, :, :])
                cd_all = wp.tile([128, 2 * NDT * DCONV * 128], BF16, tag="cdall")
                nc.sync.dma_start(cd_all[:], conv_dg[l, :, :])
                dd_all = wp.tile([128, 2 * NDT * 128], BF16, tag="ddall")
                nc.sync.dma_start(dd_all[:], d_dg[l, :, :])
                winT = [[win_all[:, (hl * 8 + kt) * 512:(hl * 8 + kt + 1) * 512]
                         for kt in range(8)] for hl in range(2)]
                woutT = [[wout_all[:, (hl * NDT + j) * DM:(hl * NDT + j + 1) * DM]
                          for j in range(NDT)] for hl in range(2)]
                wxpT = [[wxp_all[:, (hl * NDT + j) * 96:(hl * NDT + j + 1) * 96]
                         for j in range(NDT)] for hl in range(2)]
                wdtT = [wdt_all[:, hl * DL:(hl + 1) * DL] for hl in range(2)]
                cdg = [[[cd_all[:, ((hl * NDT + j) * DCONV + k) * 128:
                                ((hl * NDT + j) * DCONV + k + 1) * 128]
                         for k in range(DCONV)] for j in range(NDT)] for hl in range(2)]
                ddg = [[dd_all[:, (hl * NDT + j) * 128:(hl * NDT + j + 1) * 128]
                        for j in range(NDT)] for hl in range(2)]
                convb, dtb, Asb = [], [], []
                for j in range(NDT):
                    cb = wp.tile([128, 1], F32, tag=f"convb{j}")
                    nc.sync.dma_start(cb[:], conv_b[l, j * 128:(j + 1) * 128, :])
                    convb.append(cb)
                    db = wp.tile([128, 1], F32, tag=f"dtb{j}")
                    nc.sync.dma_start(db[:], dt_b[l, j * 128:(j + 1) * 128, :])
                    dtb.append(db)
                    at = wp.tile([128, DS], F32, tag=f"alog{j}")
                    nc.sync.dma_start(at[:], a_log[l, j * 128:(j + 1) * 128, :])
                    ae = wp.tile([128, DS], F32, tag=f"aexp{j}")
                    nc.scalar.activation(ae[:], at[:], AF.Exp)
                    an = wp.tile([128, DS], F32, tag=f"aneg{j}")
                    nc.vector.tensor_scalar_mul(an[:], ae[:], -1.0)
                    Asb.append(an)
                if apply_norm_w:
                    nw_sb = wp.tile([128, DM], F32, tag="nwsb")
                    nc.sync.dma_start(nw_sb[:], nwb[l, :, :])
                if apply_norm_b:
                    nb_sb = wp.tile([128, DM], F32, tag="nbsb")
                    nc.sync.dma_start(nb_sb[:], nbb[l, :, :])

                # per-layer resident SBUF tensors
                u_sb = [wp.tile([128, T], BF16, tag=f"u{j}", name=f"u{l}_{j}")
                        for j in range(NDT)]
                z_sb = [wp.tile([128, T], BF16, tag=f"z{j}", name=f"z{l}_{j}")
                        for j in range(NDT)]

                # DRAM staging for collectives
                xdbl_in = [dram.tile([96, L], BF16, tag=f"xdbli{b}",
                                     name=f"xdbli{l}_{b}") for b in range(B)]
                xdbl_sh = [dram.tile([96, L], BF16, tag=f"xdblo{b}", addr_space="Shared",
                                     name=f"xdblo{l}_{b}") for b in range(B)]
                odt = F32 if l == DEPTH - 1 else BF16
                out_part = [[dram.tile([LH, DM], odt, tag=f"opart{l}_{b}_{h}",
                                       name=f"opart{l}_{b}_{h}") for h in range(2)]
                            for b in range(B)]
                hred = [[dram.tile([LH, DM], odt, tag=f"hred{l}_{b}_{h}",
                                   addr_space="Shared",
                                   name=f"hred{l}_{b}_{h}") for h in range(2)]
                        for b in range(B)]
                hred_ap = [[hred[b][h].opt() for h in range(2)] for b in range(B)]
                if l == DEPTH - 1:
                    final_hred = hred

                # ================= phase A: LN + transpose + in_proj + conv ===========
                prev_ue = [None, None]
                for ci in range(NCH):
                    b = ci // 4
                    tok0 = ci * 512
                    xa_t = [], 
                    xa_t = []
                    var4 = lnp.tile([128, 4], F32, tag="var4", bufs=2)
                    bna8 = lnp.tile([128, 8], F32, tag="bna8", bufs=2)
                    # stats on DVE: bn_stats (2x 512) + bn_aggr per t-tile
                    for tti in range(4):
                        row0 = (ci % 4) * 512 + tti * 128
                        xa = lnp.tile([128, DM], BF16, tag="xa", bufs=4)
                        nc.sync.dma_start(xa[:], hget(b, row0))
                        xa_t.append(xa)
                        bst = lnp.tile([128, 12], F32, tag="bst", bufs=2)
                        nc.vector.bn_stats(bst[:, 0:6], xa[:, 0:512])
                        nc.vector.bn_stats(bst[:, 6:12], xa[:, 512:1024])
                        nc.vector.bn_aggr(bna8[:, 2 * tti:2 * tti + 2], bst[:])
                        nc.vector.tensor_copy(var4[:, tti:tti + 1],
                                              bna8[:, 2 * tti + 1:2 * tti + 2])
                    # rstd: one Sqrt per chunk -- single table-swap cluster
                    std4 = lnp.tile([128, 4], F32, tag="std4", bufs=2)
                    nc.scalar.activation(std4[:], var4[:], AF.Sqrt, bias=eps_sb[:])
                    rstd4 = lnp.tile([128, 4], F32, tag="rstd4", bufs=2)
                    nc.vector.reciprocal(rstd4[:], std4[:])
                    hn_pack = lnp.tile([128, 4096], BF16, tag="hnpack", bufs=1)
                    for tti in range(4):
                        mean = bna8[:, 2 * tti:2 * tti + 1]
                        rstd = rstd4[:, tti:tti + 1]
                        nbias = lnp.tile([128, 1], F32, tag="nbias", bufs=4)
                        nc.vector.scalar_tensor_tensor(
                            nbias[:], mean, -1.0, rstd, ALU.mult, ALU.mult
                        )
                        hcol = hn_pack[:, tti * DM:(tti + 1) * DM]
                        if apply_norm_w or apply_norm_b:
                            hn0 = lnp.tile([128, DM], F32, tag="hn0", bufs=2)
                            nc.vector.tensor_scalar(
                                hn0[:], xa_t[tti][:], rstd, nbias[:],
                                ALU.mult, ALU.add,
                            )
                            if apply_norm_w and apply_norm_b:
                                hn1 = lnp.tile([128, DM], F32, tag="hn1", bufs=2)
                                nc.vector.tensor_mul(hn1[:], hn0[:], nw_sb[:])
                                nc.vector.tensor_add(hcol, hn1[:], nb_sb[:])
                            elif apply_norm_w:
                                nc.vector.tensor_mul(hcol, hn0[:], nw_sb[:])
                            else:
                                nc.vector.tensor_add(hcol, hn0[:], nb_sb[:])
                        else:
                            nc.vector.tensor_scalar(
                                hcol, xa_t[tti][:], rstd, nbias[:],
                                ALU.mult, ALU.add,
                            )
                    # transpose via DMA xbar: hnT[p, kt, t] = hn_pack[t', kt*128+p]
                    hnT = lnp.tile([128, 8, 512], BF16, tag="hnT", bufs=1)
                    for tti in range(4):
                        nc.sync.dma_start_transpose(
                            hnT[:, :, tti * 128:(tti + 1) * 128],
                            hn_pack[:, tti * DM:(tti + 1) * DM],
                        )
                    # in_proj + conv + silu + x_proj
                    for mt in range(4):
                        pm = psA.tile([128, 512], F32, tag="pm")
                        for hl in range(2):
                            for kt in range(8):
                                nc.tensor.matmul(
                                    pm[:],
                                    winT[hl][kt][:, mt * 128:(mt + 1) * 128],
                                    hnT[:, kt, :],
                                    start=(hl == 0 and kt == 0),
                                    stop=(hl == 1 and kt == 7),
                                )
                        if mt < NDT:
                            j = mt
                            ue = sp.tile([128, 515], BF16, tag=f"ue{j}", bufs=2)
                            if ci % 4 == 0:
                                nc.vector.memset(ue[:, 0:3], 0.0)
                            else:
                                nc.vector.tensor_copy(
                                    ue[:, 0:3], prev_ue[j][:, 512:515]
                                )
                            nc.scalar.copy(ue[:, 3:515], pm[:])
                            prev_ue[j] = ue
                            pcv = psA.tile([128, 512], F32, tag="pm")
                            for hl in range(2):
                                for k in range(DCONV):
                                    nc.tensor.matmul(
                                        pcv[:], cdg[hl][j][k][:], ue[:, k:k + 512],
                                        start=(hl == 0 and k == 0),
                                        stop=(hl == 1 and k == DCONV - 1),
                                    )
                            nc.scalar.activation(
                                u_sb[j][:, tok0:tok0 + 512], pcv[:],
                                AF.Silu, bias=convb[j][:],
                            )
                        else:
                            j = mt - NDT
                            nc.scalar.activation(
                                z_sb[j][:, tok0:tok0 + 512], pm[:], AF.Silu
                            )
                    px = psA.tile([96, 512], F32, tag="pm")
                    for hl in range(2):
                        for j in range(NDT):
                            nc.tensor.matmul(
                                px[:], wxpT[hl][j][:], u_sb[j][:, tok0:tok0 + 512],
                                start=(hl == 0 and j == 0),
                                stop=(hl == 1 and j == NDT - 1),
                            )
                    xdc = sp.tile([96, 512], BF16, tag="xdc", bufs=2)
                    nc.scalar.copy(xdc[:], px[:])
                    ctok = (ci % 4) * 512
                    nc.sync.dma_start(xdbl_in[b][:, ctok:ctok + 512], xdc[:])

                    if ci == 3:
                        all_reduce(xdbl_in[0].opt(), xdbl_sh[0].opt())

                # ============= phase D: dt + scan; phase E: out_proj =============
                for b in range(B):
                    if b == 1:
                        all_reduce(xdbl_in[1].opt(), xdbl_sh[1].opt())
                    xrd = sp.tile([DTR, L], BF16, tag="xrd")
                    nc.sync.dma_start(xrd[:], xdbl_sh[b][0:DTR, :])
                    dts, dus = [], []
                    for j in range(NDT):
                        dt_j = dp.tile([128, L], BF16, tag=f"dt{j}", bufs=1,
                                       name=f"dt{l}_{b}_{j}")
                        evs = []
                        for q in range(4):
                            pdm = psS.tile([128, 512], F32, tag="ps")
                            for hl in range(2):
                                nc.tensor.matmul(
                                    pdm[:],
                                    wdtT[hl][:, j * 128:(j + 1) * 128],
                                    xrd[:, q * 512:(q + 1) * 512],
                                    start=(hl == 0), stop=(hl == 1),
                                )
                            ev = sp.tile([128, 512], F32, tag="ev", bufs=4)
                            nc.scalar.activation(ev[:], pdm[:], AF.Exp, bias=dtb[j][:])
                            evs.append(ev)
                        for q in range(4):
                            nc.scalar.activation(
                                dt_j[:, q * 512:(q + 1) * 512], evs[q][:],
                                AF.Ln, bias=one_sb[:],
                            )
                        du_j = dp.tile([128, L], BF16, tag=f"du{j}", bufs=1,
                                       name=f"du{l}_{b}_{j}")
                        nc.gpsimd.tensor_tensor(
                            du_j[:], dt_j[:], u_sb[j][:, b * L:(b + 1) * L], ALU.mult
                        )
                        dts.append(dt_j)
                        dus.append(du_j)
                    for j in range(NDT):
                        y_ps = psY.tile([128, L], F32, tag="yps")
                        for n in range(DS):
                            pb = bbp.tile([128, L], BF16, tag="pb")
                            nc.sync.dma_start(
                                pb[:],
                                xdbl_sh[b][DTR + n:DTR + n + 1, :].to_broadcast((128, L)),
                            )
                            pc = bcp.tile([128, L], BF16, tag="pc")
                            nc.sync.dma_start(
                                pc[:],
                                xdbl_sh[b][DTR + DS + n:DTR + DS + n + 1, :]
                                .to_broadcast((128, L)),
                            )
                            ada = dp.tile([128, L], BF16, tag="ada")
                            nc.scalar.activation(
                                ada[:], dts[j][:], AF.Exp, scale=Asb[j][:, n:n + 1]
                            )
                            bt = dp.tile([128, L], BF16, tag="bt")
                            if n in BT_POOL_N:
                                nc.gpsimd.tensor_tensor(bt[:], dus[j][:], pb[:], ALU.mult)
                            else:
                                nc.vector.tensor_mul(bt[:], dus[j][:], pb[:])
                            hs = dp.tile([128, L], BF16, tag="hs")
                            nc.vector.tensor_tensor_scan(
                                hs[:], ada[:], bt[:], 0.0, ALU.mult, ALU.add
                            )
                            if n in YT_POOL_N:
                                nc.gpsimd.tensor_tensor(hs[:], hs[:], pc[:], ALU.mult)
                            else:
                                nc.vector.tensor_mul(hs[:], hs[:], pc[:])
                            for q in range(4):
                                nc.tensor.matmul(
                                    y_ps[:, q * 512:(q + 1) * 512],
                                    ident_sb[:],
                                    hs[:, q * 512:(q + 1) * 512],
                                    start=(n == 0), stop=False,
                                )
                        # D*u skip term closes the accumulation
                        for hl in range(2):
                            for q in range(4):
                                nc.tensor.matmul(
                                    y_ps[:, q * 512:(q + 1) * 512],
                                    ddg[hl][j][:],
                                    u_sb[j][:, b * L + q * 512: b * L + (q + 1) * 512],
                                    start=False, stop=(hl == 1),
                                )
                        y2 = dp.tile([128, L], BF16, tag=f"y2{j}", bufs=1)
                        nc.vector.tensor_mul(
                            y2[:], y_ps[:], z_sb[j][:, b * L:(b + 1) * L]
                        )
                        dts[j] = None
                        if j == 0:
                            y2s = [y2]
                        else:
                            y2s.append(y2)
                    # phase E: out_proj in token halves, AllReduce each half
                    for h in range(2):
                        for tt in range(8):
                            t0 = h * LH + tt * 128
                            for nt in range(2):
                                po = psS.tile([128, 512], F32, tag="ps")
                                for hl in range(2):
                                    for j in range(NDT):
                                        nc.tensor.matmul(
                                            po[:],
                                            y2s[j][:, t0:t0 + 128],
                                            woutT[hl][j][:, nt * 512:(nt + 1) * 512],
                                            start=(hl == 0 and j == 0),
                                            stop=(hl == 1 and j == NDT - 1),
                                        )
                                oc = sp.tile([128, 512], odt, tag="oc", bufs=3)
                                nc.scalar.copy(oc[:], po[:])
                                nc.sync.dma_start(
                                    out_part[b][h][tt * 128:(tt + 1) * 128,
                                                   nt * 512:(nt + 1) * 512],
                                    oc[:],
                                )
                        all_reduce(out_part[b][h].opt(), hred_ap[b][h])

                def mk_hget(hred_l):
                    def _g(b, row0):
                        h = row0 // LH
                        r = row0 % LH
                        return hred_l[b][h][r:r + 128, :]
                    return _g

                hget = mk_hget(hred)

            # final: straight DRAM->DRAM copy of the f32 last-layer result
            for b in range(B):
                for h in range(2):
                    nc.sync.dma_start(
                        out_dram[b * L + h * LH: b * L + (h + 1) * LH, :],
                        final_hred[b][h][:, :],
                    )

    nc.compile()
    return nc


_CACHE = {}


def _get_nc(apply_norm_w, apply_norm_b, fake_cc=False):
    key = (apply_norm_w, apply_norm_b, fake_cc)
    if key not in _CACHE:
        _CACHE[key] = build_nc(apply_norm_w, apply_norm_b, fake_cc)
    return _CACHE[key]


def make_in_maps(x, norm_w, norm_b, in_proj_w, conv_w, conv_b, x_proj_w,
                 dt_proj_w, dt_proj_b, A_log, D, out_proj_w,
                 apply_norm_w, apply_norm_b):
    bf = mybir.dt.np(BF16)
    f = lambda a: np.ascontiguousarray(np.asarray(a), dtype=np.float32)
    fb = lambda a: np.ascontiguousarray(np.asarray(a, dtype=np.float32).astype(bf))

    def hilo(a):
        a = np.asarray(a, dtype=np.float32)
        hi = a.astype(bf)
        lo = (a - hi.astype(np.float32)).astype(bf)
        return np.ascontiguousarray(np.stack([hi, lo], axis=0))

    def pack_in(a):  # [2, DEPTH, DM, 512] -> [DEPTH, 128, 2*8*512]
        a = a.reshape(2, DEPTH, 8, 128, 512)
        return np.ascontiguousarray(
            a.transpose(1, 3, 0, 2, 4).reshape(DEPTH, 128, 2 * 8 * 512))

    def pack_pj(a, w):  # [2, DEPTH, DL, w] -> [DEPTH, 128, 2*NDT*w]
        a = a.reshape(2, DEPTH, NDT, 128, w)
        return np.ascontiguousarray(
            a.transpose(1, 3, 0, 2, 4).reshape(DEPTH, 128, 2 * NDT * w))

    def pack_dt(a):  # [2, DEPTH, DTR, DL] -> [DEPTH, DTR, 2*DL]
        return np.ascontiguousarray(
            a.transpose(1, 2, 0, 3).reshape(DEPTH, DTR, 2 * DL))

    def pack_cd(a):  # [2, DEPTH, NDT, DCONV, 128, 128] -> [DEPTH, 128, 2*NDT*DCONV*128]
        return np.ascontiguousarray(
            a.transpose(1, 4, 0, 2, 3, 5).reshape(DEPTH, 128, 2 * NDT * DCONV * 128))

    def pack_dd(a):  # [2, DEPTH, NDT, 128, 128] -> [DEPTH, 128, 2*NDT*128]
        return np.ascontiguousarray(
            a.transpose(1, 3, 0, 2, 4).reshape(DEPTH, 128, 2 * NDT * 128))

    x_tm = fb(np.asarray(x).reshape(T, DM))
    in_proj_w = np.asarray(in_proj_w)
    conv_w = np.asarray(conv_w)
    D_np = np.asarray(D)
    in_maps = []
    for c in range(NCORES):
        sl = slice(c * DL, (c + 1) * DL)
        w_in_rows = np.concatenate(
            [in_proj_w[:, sl, :], in_proj_w[:, DI + c * DL: DI + (c + 1) * DL, :]],
            axis=1,
        )  # (DEPTH, 512, 1024)
        cdg = np.zeros((DEPTH, NDT, DCONV, 128, 128), dtype=np.float32)
        ddg = np.zeros((DEPTH, NDT, 128, 128), dtype=np.float32)
        for li in range(DEPTH):
            for j in range(NDT):
                ch = slice(c * DL + j * 128, c * DL + (j + 1) * 128)
                for k in range(DCONV):
                    np.fill_diagonal(cdg[li, j, k], conv_w[li, ch, 0, k])
                np.fill_diagonal(ddg[li, j], D_np[li, ch])
        m = {
            "x_tm": x_tm,
            "w_inT": pack_in(hilo(w_in_rows.transpose(0, 2, 1))),
            "w_outT": pack_pj(hilo(np.asarray(out_proj_w)[:, :, sl].transpose(0, 2, 1)), DM),
            "w_xpT": pack_pj(hilo(np.asarray(x_proj_w)[:, :, sl].transpose(0, 2, 1)), 96),
            "w_dtT": pack_dt(hilo(np.asarray(dt_proj_w)[:, sl, :].transpose(0, 2, 1))),
            "conv_dg": pack_cd(hilo(cdg)),
            "d_dg": pack_dd(hilo(ddg)),
            "conv_b_c": f(np.asarray(conv_b)[:, sl][..., None]),
            "dt_b_c": f(np.asarray(dt_proj_b)[:, sl][..., None]),
            "a_log_c": f(np.asarray(A_log)[:, sl, :]),
            "ident_bf": np.eye(128, dtype=np.float32).astype(bf),
        }
        if apply_norm_w:
            m["norm_w_bc"] = f(np.broadcast_to(np.asarray(norm_w)[:, None, :], (DEPTH, 128, DM)))
        if apply_norm_b:
            m["norm_b_bc"] = f(np.broadcast_to(np.asarray(norm_b)[:, None, :], (DEPTH, 128, DM)))
        in_maps.append(m)
    return in_maps


def kernel(x, x_size, norm_w, norm_b, in_proj_w, conv_w, conv_b, x_proj_w,
           dt_proj_w, dt_proj_b, A_log, D, out_proj_w, **_unused):
    apply_norm_w = not np.allclose(np.asarray(norm_w), 1.0)
    apply_norm_b = not np.allclose(np.asarray(norm_b), 0.0)
    nc = _get_nc(apply_norm_w, apply_norm_b)
    in_maps = make_in_maps(
        x, norm_w, norm_b, in_proj_w, conv_w, conv_b, x_proj_w,
        dt_proj_w, dt_proj_b, A_log, D, out_proj_w,
        apply_norm_w, apply_norm_b,
    )
    res = run_bass_kernel_spmd(nc, in_maps, core_ids=list(range(NCORES)))
    return res.results[0]["out_tm"].reshape(B, L, DM).astype(np.float32)


# revision 21
# speedup vs baseline: 1.2875x; 1.0940x over previous
"""Trainium2 Bass kernel for a 2-layer Mamba stack (BasicLayer). v2.

Per layer: LayerNorm -> in_proj (1024->4096) -> causal depthwise conv(k=4)
+ SiLU -> x_proj (2048->96) -> dt_proj + softplus -> selective scan over
L=2048 -> gate with SiLU(z) -> out_proj (2048->1024).

Sharding: tensor-parallel over d_inner (2048 / 8 cores = 256 channels per
core).  Cross-core sums (x_proj and out_proj contractions) are AllReduced
on-chip in bf16, out_proj split in token halves so the collectives overlap
compute.  All matmuls run in bf16 (fp32 matmul is 4 cycles/row vs 1 for
bf16).  Transposes use the DMA xbar (dma_start_transpose) instead of the
PE+PSUM path.  The depthwise conv and the D*u skip term are expressed as
diagonal-matrix matmuls on the PE so the vector engine only carries the
scan itself plus the B/C elementwise products.  softplus(x) is computed as
Ln(1+Exp(x)) -- both functions live in the same activation table, and the
LayerNorm rstd is the only per-chunk table swap (Sqrt).
"""

import numpy as np

try:
    import concourse.bass as bass
except ImportError:  # pragma: no cover
    import sys

    sys.path.insert(0, "/opt/trn_rl_repo")
    import concourse.bass as bass

import concourse.bacc as bacc
import concourse.mybir as mybir
import concourse.tile as tile
from concourse.bass_utils import run_bass_kernel_spmd

F32 = mybir.dt.float32
BF16 = mybir.dt.bfloat16
AF = mybir.ActivationFunctionType
ALU = mybir.AluOpType

B, L = 2, 2048
DM, DI, DS, DTR, DCONV, DEPTH = 1024, 2048, 16, 64, 4, 2
EPS = 1e-5
NCORES = 8
DL = DI // NCORES          # 256 channels per core
NDT = DL // 128            # 2 channel tiles per core
T = B * L                  # 4096 tokens
NCH = T // 512             # 8 chunks of 512 tokens
LH = L // 2                # token half for out AllReduce chunking

# knobs
YT_POOL_N = set()          # scan ns whose yt-mul runs on gpsimd instead of DVE
BT_POOL_N = {1, 3, 5, 7, 9, 11, 13, 15}


def build_nc(apply_norm_w: bool, apply_norm_b: bool, fake_cc: bool = False):
    nc = bacc.Bacc(
        "TRN2",
        target_bir_lowering=False,
        debug=False,
        enable_asserts=False,
        num_devices=NCORES,
    )

    # ---- I/O declarations (per-core data supplied via in_maps) ----
    x_dram = nc.dram_tensor("x_tm", [T, DM], BF16, kind="ExternalInput")
    # packed per-layer weights: big contiguous blocks to minimize DMA count
    w_inT = nc.dram_tensor("w_inT", [DEPTH, 128, 2 * 8 * 512], BF16, kind="ExternalInput")
    w_outT = nc.dram_tensor("w_outT", [DEPTH, 128, 2 * NDT * DM], BF16, kind="ExternalInput")
    w_xpT = nc.dram_tensor("w_xpT", [DEPTH, 128, 2 * NDT * 96], BF16, kind="ExternalInput")
    w_dtT = nc.dram_tensor("w_dtT", [DEPTH, DTR, 2 * DL], BF16, kind="ExternalInput")
    conv_dg = nc.dram_tensor("conv_dg", [DEPTH, 128, 2 * NDT * DCONV * 128], BF16,
                             kind="ExternalInput")
    d_dg = nc.dram_tensor("d_dg", [DEPTH, 128, 2 * NDT * 128], BF16, kind="ExternalInput")
    conv_b = nc.dram_tensor("conv_b_c", [DEPTH, DL, 1], F32, kind="ExternalInput")
    dt_b = nc.dram_tensor("dt_b_c", [DEPTH, DL, 1], F32, kind="ExternalInput")
    a_log = nc.dram_tensor("a_log_c", [DEPTH, DL, DS], F32, kind="ExternalInput")
    ident = nc.dram_tensor("ident_bf", [128, 128], BF16, kind="ExternalInput")
    if apply_norm_w:
        nwb = nc.dram_tensor("norm_w_bc", [DEPTH, 128, DM], F32, kind="ExternalInput")
    if apply_norm_b:
        nbb = nc.dram_tensor("norm_b_bc", [DEPTH, 128, DM], F32, kind="ExternalInput")
    out_dram = nc.dram_tensor("out_tm", [T, DM], F32, kind="ExternalOutput")

    groups = [list(range(NCORES))]

    def all_reduce(src_ap, dst_ap):
        if fake_cc:
            nc.sync.dma_start(dst_ap, src_ap)
        else:
            nc.gpsimd.collective_compute(
                "AllReduce", ALU.add, replica_groups=groups,
                ins=[src_ap], outs=[dst_ap],
            )

    with tile.TileContext(nc, num_cores=NCORES) as tc:
        with (
            tc.tile_pool(name="wp", bufs=1) as wp,
            tc.tile_pool(name="lnp", bufs=2) as lnp,
            tc.tile_pool(name="sp", bufs=2) as sp,
            tc.tile_pool(name="dp", bufs=2) as dp,
            tc.tile_pool(name="bbp", bufs=3) as bbp,
            tc.tile_pool(name="bcp", bufs=3) as bcp,
            tc.tile_pool(name="psA", bufs=2, space="PSUM") as psA,
            tc.tile_pool(name="psY", bufs=1, space="PSUM") as psY,
            tc.tile_pool(name="psS", bufs=2, space="PSUM") as psS,
            tc.tile_pool(name="dram", bufs=2, space="DRAM") as dram,
        ):
            ident_sb = wp.tile([128, 128], BF16, tag="ident")
            nc.sync.dma_start(ident_sb[:], ident[:, :])
            eps_sb = wp.tile([128, 1], F32, tag="eps")
            nc.vector.memset(eps_sb[:], EPS)
            one_sb = wp.tile([128, 1], F32, tag="one")
            nc.vector.memset(one_sb[:], 1.0)

            # hsrc(b, row0) -> AP of 128 input rows for this layer
            hsrc_l0 = [x_dram.ap()[0:L, :], x_dram.ap()[L:T, :]]

            def hsrc_l0_get(b, row0):
                return hsrc_l0[b][row0:row0 + 128, :]

            hget = hsrc_l0_get

            for l in range(DEPTH):
                # ---- per-layer weights ----
                win_all = wp.tile([128, 2 * 8 * 512], BF16, tag="winall")
                nc.sync.dma_start(win_all[:], w_inT[l, :, :])
                wout_all = wp.tile([128, 2 * NDT * DM], BF16, tag="woutall")
                nc.sync.dma_start(wout_all[:], w_outT[l, :, :])
                wxp_all = wp.tile([128, 2 * NDT * 96], BF16, tag="wxpall")
                nc.sync.dma_start(wxp_all[:], w_xpT[l, :, :])
                wdt_all = wp.tile([DTR, 2 * DL], BF16, tag="wdtall")
                nc.sync.dma_start(wdt_all[:], w_dtT[l, :, :])
                cd_all = wp.tile([128, 2 * NDT * DCONV * 128], BF16, tag="cdall")
                nc.sync.dma_start(cd_all[:], conv_dg[l, :, :])
                dd_all = wp.tile([128, 2 * NDT * 128], BF16, tag="ddall")
                nc.sync.dma_start(dd_all[:], d_dg[l, :, :])
                winT = [[win_all[:, (hl * 8 + kt) * 512:(hl * 8 + kt + 1) * 512]
                         for kt in range(8)] for hl in range(2)]
                woutT = [[wout_all[:, (hl * NDT + j) * DM:(hl * NDT + j + 1) * DM]
                          for j in range(NDT)] for hl in range(2)]
                wxpT = [[wxp_all[:, (hl * NDT + j) * 96:(hl * NDT + j + 1) * 96]
                         for j in range(NDT)] for hl in range(2)]
                wdtT = [wdt_all[:, hl * DL:(hl + 1) * DL] for hl in range(2)]
                cdg = [[[cd_all[:, ((hl * NDT + j) * DCONV + k) * 128:
                                ((hl * NDT + j) * DCONV + k + 1) * 128]
                         for k in range(DCONV)] for j in range(NDT)] for hl in range(2)]
                ddg = [[dd_all[:, (hl * NDT + j) * 128:(hl * NDT + j + 1) * 128]
                        for j in range(NDT)] for hl in range(2)]
                convb, dtb, Asb = [], [], []
                for j in range(NDT):
                    cb = wp.tile([128, 1], F32, tag=f"convb{j}")
                    nc.sync.dma_start(cb[:], conv_b[l, j * 128:(j + 1) * 128, :])
                    convb.append(cb)
                    db = wp.tile([128, 1], F32, tag=f"dtb{j}")
                    nc.sync.dma_start(db[:], dt_b[l, j * 128:(j + 1) * 128, :])
                    dtb.append(db)
                    at = wp.tile([128, DS], F32, tag=f"alog{j}")
                    nc.sync.dma_start(at[:], a_log[l, j * 128:(j + 1) * 128, :])
                    ae = wp.tile([128, DS], F32, tag=f"aexp{j}")
                    nc.scalar.activation(ae[:], at[:], AF.Exp)
                    an = wp.tile([128, DS], F32, tag=f"aneg{j}")
                    nc.vector.tensor_scalar_mul(an[:], ae[:], -1.0)
                    Asb.append(an)
                if apply_norm_w:
                    nw_sb = wp.tile([128, DM], F32, tag="nwsb")
                    nc.sync.dma_start(nw_sb[:], nwb[l, :, :])
                if apply_norm_b:
                    nb_sb = wp.tile([128, DM], F32, tag="nbsb")
                    nc.sync.dma_start(nb_sb[:], nbb[l, :, :])

                # per-layer resident SBUF tensors
                u_sb = [wp.tile([128, T], BF16, tag=f"u{j}", name=f"u{l}_{j}")
                        for j in range(NDT)]
                z_sb = [wp.tile([128, T], BF16, tag=f"z{j}", name=f"z{l}_{j}")
                        for j in range(NDT)]

                # DRAM staging for collectives
                xdbl_in = [dram.tile([96, L], BF16, tag=f"xdbli{b}",
                                     name=f"xdbli{l}_{b}") for b in range(B)]
                xdbl_sh = [dram.tile([96, L], BF16, tag=f"xdblo{b}", addr_space="Shared",
                                     name=f"xdblo{l}_{b}") for b in range(B)]
                odt = BF16
                out_part = [[dram.tile([LH, DM], odt, tag=f"opart{l}_{b}_{h}",
                                       name=f"opart{l}_{b}_{h}") for h in range(2)]
                            for b in range(B)]
                hred = [[dram.tile([LH, DM], odt, tag=f"hred{l}_{b}_{h}",
                                   addr_space="Shared",
                                   name=f"hred{l}_{b}_{h}") for h in range(2)]
                        for b in range(B)]
                hred_ap = [[hred[b][h].opt() for h in range(2)] for b in range(B)]
                if l == DEPTH - 1:
                    final_hred = hred

                # ================= phase A: LN + transpose + in_proj + conv ===========
                prev_ue = [None, None]
                for ci in range(NCH):
                    b = ci // 4
                    tok0 = ci * 512
                    xa_t = [], 
                    xa_t = []
                    var4 = lnp.tile([128, 4], F32, tag="var4", bufs=2)
                    bna8 = lnp.tile([128, 8], F32, tag="bna8", bufs=2)
                    # stats on DVE: bn_stats (2x 512) + bn_aggr per t-tile
                    for tti in range(4):
                        row0 = (ci % 4) * 512 + tti * 128
                        xa = lnp.tile([128, DM], BF16, tag="xa", bufs=4)
                        nc.sync.dma_start(xa[:], hget(b, row0))
                        xa_t.append(xa)
                        bst = lnp.tile([128, 12], F32, tag="bst", bufs=2)
                        nc.vector.bn_stats(bst[:, 0:6], xa[:, 0:512])
                        nc.vector.bn_stats(bst[:, 6:12], xa[:, 512:1024])
                        nc.vector.bn_aggr(bna8[:, 2 * tti:2 * tti + 2], bst[:])
                        nc.vector.tensor_copy(var4[:, tti:tti + 1],
                                              bna8[:, 2 * tti + 1:2 * tti + 2])
                    # rstd: one Sqrt per chunk -- single table-swap cluster
                    std4 = lnp.tile([128, 4], F32, tag="std4", bufs=2)
                    nc.scalar.activation(std4[:], var4[:], AF.Sqrt, bias=eps_sb[:])
                    rstd4 = lnp.tile([128, 4], F32, tag="rstd4", bufs=2)
                    nc.vector.reciprocal(rstd4[:], std4[:])
                    hn_pack = lnp.tile([128, 4096], BF16, tag="hnpack", bufs=1)
                    for tti in range(4):
                        mean = bna8[:, 2 * tti:2 * tti + 1]
                        rstd = rstd4[:, tti:tti + 1]
                        nbias = lnp.tile([128, 1], F32, tag="nbias", bufs=4)
                        nc.vector.scalar_tensor_tensor(
                            nbias[:], mean, -1.0, rstd, ALU.mult, ALU.mult
                        )
                        hcol = hn_pack[:, tti * DM:(tti + 1) * DM]
                        if apply_norm_w or apply_norm_b:
                            hn0 = lnp.tile([128, DM], F32, tag="hn0", bufs=2)
                            nc.vector.tensor_scalar(
                                hn0[:], xa_t[tti][:], rstd, nbias[:],
                                ALU.mult, ALU.add,
                            )
                            if apply_norm_w and apply_norm_b:
                                hn1 = lnp.tile([128, DM], F32, tag="hn1", bufs=2)
                                nc.vector.tensor_mul(hn1[:], hn0[:], nw_sb[:])
                                nc.vector.tensor_add(hcol, hn1[:], nb_sb[:])
                            elif apply_norm_w:
                                nc.vector.tensor_mul(hcol, hn0[:], nw_sb[:])
                            else:
                                nc.vector.tensor_add(hcol, hn0[:], nb_sb[:])
                        else:
                            nc.vector.tensor_scalar(
                                hcol, xa_t[tti][:], rstd, nbias[:],
                                ALU.mult, ALU.add,
                            )
                    # transpose via DMA xbar: hnT[p, kt, t] = hn_pack[t', kt*128+p]
                    hnT = lnp.tile([128, 8, 512], BF16, tag="hnT", bufs=1)
                    for tti in range(4):
                        nc.sync.dma_start_transpose(
                            hnT[:, :, tti * 128:(tti + 1) * 128],
                            hn_pack[:, tti * DM:(tti + 1) * DM],
                        )
                    # in_proj + conv + silu + x_proj
                    for mt in range(4):
                        pm = psA.tile([128, 512], F32, tag="pm")
                        for hl in range(2):
                            for kt in range(8):
                                nc.tensor.matmul(
                                    pm[:],
                                    winT[hl][kt][:, mt * 128:(mt + 1) * 128],
                                    hnT[:, kt, :],
                                    start=(hl == 0 and kt == 0),
                                    stop=(hl == 1 and kt == 7),
                                )
                        if mt < NDT:
                            j = mt
                            ue = sp.tile([128, 515], BF16, tag=f"ue{j}", bufs=2)
                            if ci % 4 == 0:
                                nc.vector.memset(ue[:, 0:3], 0.0)
                            else:
                                nc.vector.tensor_copy(
                                    ue[:, 0:3], prev_ue[j][:, 512:515]
                                )
                            nc.scalar.copy(ue[:, 3:515], pm[:])
                            prev_ue[j] = ue
                            pcv = psA.tile([128, 512], F32, tag="pm")
                            for hl in range(2):
                                for k in range(DCONV):
                                    nc.tensor.matmul(
                                        pcv[:], cdg[hl][j][k][:], ue[:, k:k + 512],
                                        start=(hl == 0 and k == 0),
                                        stop=(hl == 1 and k == DCONV - 1),
                                    )
                            nc.scalar.activation(
                                u_sb[j][:, tok0:tok0 + 512], pcv[:],
                                AF.Silu, bias=convb[j][:],
                            )
                        else:
                            j = mt - NDT
                            nc.scalar.activation(
                                z_sb[j][:, tok0:tok0 + 512], pm[:], AF.Silu
                            )
                    px = psA.tile([96, 512], F32, tag="pm")
                    for hl in range(2):
                        for j in range(NDT):
                            nc.tensor.matmul(
                                px[:], wxpT[hl][j][:], u_sb[j][:, tok0:tok0 + 512],
                                start=(hl == 0 and j == 0),
                                stop=(hl == 1 and j == NDT - 1),
                            )
                    xdc = sp.tile([96, 512], BF16, tag="xdc", bufs=2)
                    nc.scalar.copy(xdc[:], px[:])
                    ctok = (ci % 4) * 512
                    nc.sync.dma_start(xdbl_in[b][:, ctok:ctok + 512], xdc[:])

                    if ci == 3:
                        all_reduce(xdbl_in[0].opt(), xdbl_sh[0].opt())

                # ============= phase D: dt + scan; phase E: out_proj =============
                for b in range(B):
                    if b == 1:
                        all_reduce(xdbl_in[1].opt(), xdbl_sh[1].opt())
                    xrd = sp.tile([DTR, L], BF16, tag="xrd")
                    nc.sync.dma_start(xrd[:], xdbl_sh[b][0:DTR, :])
                    dts, dus = [], []
                    for j in range(NDT):
                        dt_j = dp.tile([128, L], BF16, tag=f"dt{j}", bufs=1,
                                       name=f"dt{l}_{b}_{j}")
                        evs = []
                        for q in range(4):
                            pdm = psS.tile([128, 512], F32, tag="ps")
                            for hl in range(2):
                                nc.tensor.matmul(
                                    pdm[:],
                                    wdtT[hl][:, j * 128:(j + 1) * 128],
                                    xrd[:, q * 512:(q + 1) * 512],
                                    start=(hl == 0), stop=(hl == 1),
                                )
                            ev = sp.tile([128, 512], F32, tag="ev", bufs=4)
                            nc.scalar.activation(ev[:], pdm[:], AF.Exp, bias=dtb[j][:])
                            evs.append(ev)
                        for q in range(4):
                            nc.scalar.activation(
                                dt_j[:, q * 512:(q + 1) * 512], evs[q][:],
                                AF.Ln, bias=one_sb[:],
                            )
                        du_j = dp.tile([128, L], BF16, tag=f"du{j}", bufs=1,
                                       name=f"du{l}_{b}_{j}")
                        nc.gpsimd.tensor_tensor(
                            du_j[:], dt_j[:], u_sb[j][:, b * L:(b + 1) * L], ALU.mult
                        )
                        dts.append(dt_j)
                        dus.append(du_j)
                    for j in range(NDT):
                        y_ps = psY.tile([128, L], F32, tag="yps")
                        for n in range(DS):
                            pb = bbp.tile([128, L], BF16, tag="pb")
                            nc.sync.dma_start(
                                pb[:],
                                xdbl_sh[b][DTR + n:DTR + n + 1, :].to_broadcast((128, L)),
                            )
                            pc = bcp.tile([128, L], BF16, tag="pc")
                            nc.sync.dma_start(
                                pc[:],
                                xdbl_sh[b][DTR + DS + n:DTR + DS + n + 1, :]
                                .to_broadcast((128, L)),
                            )
                            ada = dp.tile([128, L], BF16, tag="ada")
                            nc.scalar.activation(
                                ada[:], dts[j][:], AF.Exp, scale=Asb[j][:, n:n + 1]
                            )
                            bt = dp.tile([128, L], BF16, tag="bt")
                            if n in BT_POOL_N:
                                nc.gpsimd.tensor_tensor(bt[:], dus[j][:], pb[:], ALU.mult)
                            else:
                                nc.vector.tensor_mul(bt[:], dus[j][:], pb[:])
                            hs = dp.tile([128, L], BF16, tag="hs")
                            nc.vector.tensor_tensor_scan(
                                hs[:], ada[:], bt[:], 0.0, ALU.mult, ALU.add
                            )
                            if n in YT_POOL_N:
                                nc.gpsimd.tensor_tensor(hs[:], hs[:], pc[:], ALU.mult)
                            else:
                                nc.vector.tensor_mul(hs[:], hs[:], pc[:])
                            for q in range(4):
                                nc.tensor.matmul(
                                    y_ps[:, q * 512:(q + 1) * 512],
                                    ident_sb[:],
                                    hs[:, q * 512:(q + 1) * 512],
                                    start=(n == 0), stop=False,
                                )
                        # D*u skip term closes the accumulation
                        for hl in range(2):
                            for q in range(4):
                                nc.tensor.matmul(
                                    y_ps[:, q * 512:(q + 1) * 512],
                                    ddg[hl][j][:],
                                    u_sb[j][:, b * L + q * 512: b * L + (q + 1) * 512],
                                    start=False, stop=(hl == 1),
                                )
                        y2 = dp.tile([128, L], BF16, tag=f"y2{j}", bufs=1)
                        nc.vector.tensor_mul(
                            y2[:], y_ps[:], z_sb[j][:, b * L:(b + 1) * L]
                        )
                        dts[j] = None
                        if j == 0:
                            y2s = [y2]
                        else:
                            y2s.append(y2)
                    # phase E: out_proj in token halves, AllReduce each half
                    for h in range(2):
                        for tt in range(8):
                            t0 = h * LH + tt * 128
                            for nt in range(2):
                                po = psS.tile([128, 512], F32, tag="ps")
                                for hl in range(2):
                                    for j in range(NDT):
                                        nc.tensor.matmul(
                                            po[:],
                                            y2s[j][:, t0:t0 + 128],
                                            woutT[hl][j][:, nt * 512:(nt + 1) * 512],
                                            start=(hl == 0 and j == 0),
                                            stop=(hl == 1 and j == NDT - 1),
                                        )
                                oc = sp.tile([128, 512], odt, tag="oc", bufs=3)
                                nc.scalar.copy(oc[:], po[:])
                                nc.sync.dma_start(
                                    out_part[b][h][tt * 128:(tt + 1) * 128,
                                                   nt * 512:(nt + 1) * 512],
                                    oc[:],
                                )
                        all_reduce(out_part[b][h].opt(), hred_ap[b][h])

                def mk_hget(hred_l):
                    def _g(b, row0):
                        h = row0 // LH
                        r = row0 % LH
                        return hred_l[b][h][r:r + 128, :]
                    return _g

                hget = mk_hget(hred)

            # final: cast bf16 -> f32 and store
            for b in range(B):
                for h in range(2):
                    for rt in range(LH // 128):
                        ld = sp.tile([128, DM], BF16, tag="fld", bufs=2)
                        nc.sync.dma_start(
                            ld[:], final_hred[b][h][rt * 128:(rt + 1) * 128, :])
                        fc = sp.tile([128, DM], F32, tag="ffc", bufs=2)
                        if rt % 2 == 0:
                            nc.scalar.copy(fc[:], ld[:])
                        else:
                            nc.vector.tensor_copy(fc[:], ld[:])
                        nc.sync.dma_start(
                            out_dram[b * L + h * LH + rt * 128:
                                     b * L + h * LH + (rt + 1) * 128, :],
                            fc[:],
                        )

    nc.compile()
    return nc


_CACHE = {}


def _get_nc(apply_norm_w, apply_norm_b, fake_cc=False):
    key = (apply_norm_w, apply_norm_b, fake_cc)
    if key not in _CACHE:
        _CACHE[key] = build_nc(apply_norm_w, apply_norm_b, fake_cc)
    return _CACHE[key]


def make_in_maps(x, norm_w, norm_b, in_proj_w, conv_w, conv_b, x_proj_w,
                 dt_proj_w, dt_proj_b, A_log, D, out_proj_w,
                 apply_norm_w, apply_norm_b):
    bf = mybir.dt.np(BF16)
    f = lambda a: np.ascontiguousarray(np.asarray(a), dtype=np.float32)
    fb = lambda a: np.ascontiguousarray(np.asarray(a, dtype=np.float32).astype(bf))

    def hilo(a):
        a = np.asarray(a, dtype=np.float32)
        hi = a.astype(bf)
        lo = (a - hi.astype(np.float32)).astype(bf)
        return np.ascontiguousarray(np.stack([hi, lo], axis=0))

    def pack_in(a):  # [2, DEPTH, DM, 512] -> [DEPTH, 128, 2*8*512]
        a = a.reshape(2, DEPTH, 8, 128, 512)
        return np.ascontiguousarray(
            a.transpose(1, 3, 0, 2, 4).reshape(DEPTH, 128, 2 * 8 * 512))

    def pack_pj(a, w):  # [2, DEPTH, DL, w] -> [DEPTH, 128, 2*NDT*w]
        a = a.reshape(2, DEPTH, NDT, 128, w)
        return np.ascontiguousarray(
            a.transpose(1, 3, 0, 2, 4).reshape(DEPTH, 128, 2 * NDT * w))

    def pack_dt(a):  # [2, DEPTH, DTR, DL] -> [DEPTH, DTR, 2*DL]
        return np.ascontiguousarray(
            a.transpose(1, 2, 0, 3).reshape(DEPTH, DTR, 2 * DL))

    def pack_cd(a):  # [2, DEPTH, NDT, DCONV, 128, 128] -> [DEPTH, 128, 2*NDT*DCONV*128]
        return np.ascontiguousarray(
            a.transpose(1, 4, 0, 2, 3, 5).reshape(DEPTH, 128, 2 * NDT * DCONV * 128))

    def pack_dd(a):  # [2, DEPTH, NDT, 128, 128] -> [DEPTH, 128, 2*NDT*128]
        return np.ascontiguousarray(
            a.transpose(1, 3, 0, 2, 4).reshape(DEPTH, 128, 2 * NDT * 128))

    x_tm = fb(np.asarray(x).reshape(T, DM))
    in_proj_w = np.asarray(in_proj_w)
    conv_w = np.asarray(conv_w)
    D_np = np.asarray(D)
    in_maps = []
    for c in range(NCORES):
        sl = slice(c * DL, (c + 1) * DL)
        w_in_rows = np.concatenate(
            [in_proj_w[:, sl, :], in_proj_w[:, DI + c * DL: DI + (c + 1) * DL, :]],
            axis=1,
        )  # (DEPTH, 512, 1024)
        cdg = np.zeros((DEPTH, NDT, DCONV, 128, 128), dtype=np.float32)
        ddg = np.zeros((DEPTH, NDT, 128, 128), dtype=np.float32)
        for li in range(DEPTH):
            for j in range(NDT):
                ch = slice(c * DL + j * 128, c * DL + (j + 1) * 128)
                for k in range(DCONV):
                    np.fill_diagonal(cdg[li, j, k], conv_w[li, ch, 0, k])
                np.fill_diagonal(ddg[li, j], D_np[li, ch])
        m = {
            "x_tm": x_tm,
            "w_inT": pack_in(hilo(w_in_rows.transpose(0, 2, 1))),
            "w_outT": pack_pj(hilo(np.asarray(out_proj_w)[:, :, sl].transpose(0, 2, 1)), DM),
            "w_xpT": pack_pj(hilo(np.asarray(x_proj_w)[:, :, sl].transpose(0, 2, 1)), 96),
            "w_dtT": pack_dt(hilo(np.asarray(dt_proj_w)[:, sl, :].transpose(0, 2, 1))),
            "conv_dg": pack_cd(hilo(cdg)),
            "d_dg": pack_dd(hilo(ddg)),
            "conv_b_c": f(np.asarray(conv_b)[:, sl][..., None]),
            "dt_b_c": f(np.asarray(dt_proj_b)[:, sl][..., None]),
            "a_log_c": f(np.asarray(A_log)[:, sl, :]),
            "ident_bf": np.eye(128, dtype=np.float32).astype(bf),
        }
        if apply_norm_w:
            m["norm_w_bc"] = f(np.broadcast_to(np.asarray(norm_w)[:, None, :], (DEPTH, 128, DM)))
        if apply_norm_b:
            m["norm_b_bc"] = f(np.broadcast_to(np.asarray(norm_b)[:, None, :], (DEPTH, 128, DM)))
        in_maps.append(m)
    return in_maps


def kernel(x, x_size, norm_w, norm_b, in_proj_w, conv_w, conv_b, x_proj_w,
           dt_proj_w, dt_proj_b, A_log, D, out_proj_w, **_unused):
    apply_norm_w = not np.allclose(np.asarray(norm_w), 1.0)
    apply_norm_b = not np.allclose(np.asarray(norm_b), 0.0)
    nc = _get_nc(apply_norm_w, apply_norm_b)
    in_maps = make_in_maps(
        x, norm_w, norm_b, in_proj_w, conv_w, conv_b, x_proj_w,
        dt_proj_w, dt_proj_b, A_log, D, out_proj_w,
        apply_norm_w, apply_norm_b,
    )
    res = run_bass_kernel_spmd(nc, in_maps, core_ids=list(range(NCORES)))
    return res.results[0]["out_tm"].reshape(B, L, DM).astype(np.float32)


# revision 22
# speedup vs baseline: 1.5017x; 1.1664x over previous
"""Trainium2 Bass kernel for a 2-layer Mamba stack (BasicLayer). v2.

Per layer: LayerNorm -> in_proj (1024->4096) -> causal depthwise conv(k=4)
+ SiLU -> x_proj (2048->96) -> dt_proj + softplus -> selective scan over
L=2048 -> gate with SiLU(z) -> out_proj (2048->1024).

Sharding: tensor-parallel over d_inner (2048 / 8 cores = 256 channels per
core).  Cross-core sums (x_proj and out_proj contractions) are AllReduced
on-chip in bf16, out_proj split in token halves so the collectives overlap
compute.  All matmuls run in bf16 (fp32 matmul is 4 cycles/row vs 1 for
bf16).  Transposes use the DMA xbar (dma_start_transpose) instead of the
PE+PSUM path.  The depthwise conv and the D*u skip term are expressed as
diagonal-matrix matmuls on the PE so the vector engine only carries the
scan itself plus the B/C elementwise products.  softplus(x) is computed as
Ln(1+Exp(x)) -- both functions live in the same activation table, and the
LayerNorm rstd is the only per-chunk table swap (Sqrt).
"""

import numpy as np

try:
    import concourse.bass as bass
except ImportError:  # pragma: no cover
    import sys

    sys.path.insert(0, "/opt/trn_rl_repo")
    import concourse.bass as bass

import concourse.bacc as bacc
import concourse.mybir as mybir
import concourse.tile as tile
from concourse.bass_utils import run_bass_kernel_spmd

F32 = mybir.dt.float32
BF16 = mybir.dt.bfloat16
AF = mybir.ActivationFunctionType
ALU = mybir.AluOpType

B, L = 2, 2048
DM, DI, DS, DTR, DCONV, DEPTH = 1024, 2048, 16, 64, 4, 2
EPS = 1e-5
NCORES = 8
DL = DI // NCORES          # 256 channels per core
NDT = DL // 128            # 2 channel tiles per core
T = B * L                  # 4096 tokens
NCH = T // 512             # 8 chunks of 512 tokens
LH = L // 2                # token half for out AllReduce chunking

# knobs
YT_POOL_N = set()          # scan ns whose yt-mul runs on gpsimd instead of DVE
BT_POOL_N = {1, 3, 5, 7, 9, 11, 13, 15}


def build_nc(apply_norm_w: bool, apply_norm_b: bool, fake_cc: bool = False):
    nc = bacc.Bacc(
        "TRN2",
        target_bir_lowering=False,
        debug=False,
        enable_asserts=False,
        num_devices=NCORES,
    )

    # ---- I/O declarations (per-core data supplied via in_maps) ----
    x_dram = nc.dram_tensor("x_tm", [T, DM], BF16, kind="ExternalInput")
    # packed per-layer weights: big contiguous blocks to minimize DMA count
    w_inT = nc.dram_tensor("w_inT", [DEPTH, 128, 2 * 8 * 512], BF16, kind="ExternalInput")
    w_outT = nc.dram_tensor("w_outT", [DEPTH, 128, 2 * NDT * DM], BF16, kind="ExternalInput")
    w_xpT = nc.dram_tensor("w_xpT", [DEPTH, 128, 2 * NDT * 96], BF16, kind="ExternalInput")
    w_dtT = nc.dram_tensor("w_dtT", [DEPTH, DTR, 2 * DL], BF16, kind="ExternalInput")
    conv_dg = nc.dram_tensor("conv_dg", [DEPTH, 128, 2 * NDT * DCONV * 128], BF16,
                             kind="ExternalInput")
    d_dg = nc.dram_tensor("d_dg", [DEPTH, 128, 2 * NDT * 128], BF16, kind="ExternalInput")
    conv_b = nc.dram_tensor("conv_b_c", [DEPTH, DL, 1], F32, kind="ExternalInput")
    dt_b = nc.dram_tensor("dt_b_c", [DEPTH, DL, 1], F32, kind="ExternalInput")
    a_log = nc.dram_tensor("a_log_c", [DEPTH, DL, DS], F32, kind="ExternalInput")
    ident = nc.dram_tensor("ident_bf", [128, 128], BF16, kind="ExternalInput")
    if apply_norm_w:
        nwb = nc.dram_tensor("norm_w_bc", [DEPTH, 128, DM], F32, kind="ExternalInput")
    if apply_norm_b:
        nbb = nc.dram_tensor("norm_b_bc", [DEPTH, 128, DM], F32, kind="ExternalInput")
    out_dram = nc.dram_tensor("out_tm", [T, DM], F32, kind="ExternalOutput")

    groups = [list(range(NCORES))]

    def all_reduce(src_ap, dst_ap):
        if fake_cc:
            nc.sync.dma_start(dst_ap, src_ap)
        else:
            nc.gpsimd.collective_compute(
                "AllReduce", ALU.add, replica_groups=groups,
                ins=[src_ap], outs=[dst_ap],
            )

    with tile.TileContext(nc, num_cores=NCORES) as tc:
        with (
            tc.tile_pool(name="wp", bufs=1) as wp,
            tc.tile_pool(name="lnp", bufs=2) as lnp,
            tc.tile_pool(name="sp", bufs=2) as sp,
            tc.tile_pool(name="dp", bufs=2) as dp,
            tc.tile_pool(name="bbp", bufs=3) as bbp,
            tc.tile_pool(name="bcp", bufs=3) as bcp,
            tc.tile_pool(name="psA", bufs=2, space="PSUM") as psA,
            tc.tile_pool(name="psY", bufs=1, space="PSUM") as psY,
            tc.tile_pool(name="psS", bufs=2, space="PSUM") as psS,
            tc.tile_pool(name="dram", bufs=2, space="DRAM") as dram,
        ):
            ident_sb = wp.tile([128, 128], BF16, tag="ident")
            nc.sync.dma_start(ident_sb[:], ident[:, :])
            eps_sb = wp.tile([128, 1], F32, tag="eps")
            nc.vector.memset(eps_sb[:], EPS)
            one_sb = wp.tile([128, 1], F32, tag="one")
            nc.vector.memset(one_sb[:], 1.0)

            # hsrc(b, row0) -> AP of 128 input rows for this layer
            hsrc_l0 = [x_dram.ap()[0:L, :], x_dram.ap()[L:T, :]]

            def hsrc_l0_get(b, row0):
                return hsrc_l0[b][row0:row0 + 128, :]

            hget = hsrc_l0_get

            for l in range(DEPTH):
                # ---- per-layer weights ----
                win_all = wp.tile([128, 2 * 8 * 512], BF16, tag="winall")
                nc.sync.dma_start(win_all[:], w_inT[l, :, :])
                wout_all = wp.tile([128, 2 * NDT * DM], BF16, tag="woutall")
                nc.sync.dma_start(wout_all[:], w_outT[l, :, :])
                wxp_all = wp.tile([128, 2 * NDT * 96], BF16, tag="wxpall")
                nc.sync.dma_start(wxp_all[:], w_xpT[l, :, :])
                wdt_all = wp.tile([DTR, 2 * DL], BF16, tag="wdtall")
                nc.sync.dma_start(wdt_all[:], w_dtT[l, :, :])
                cd_all = wp.tile([128, 2 * NDT * DCONV * 128], BF16, tag="cdall")
                nc.sync.dma_start(cd_all[:], conv_dg[l, :, :])
                dd_all = wp.tile([128, 2 * NDT * 128], BF16, tag="ddall")
                nc.sync.dma_start(dd_all[:], d_dg[l, :, :])
                winT = [[win_all[:, (hl * 8 + kt) * 512:(hl * 8 + kt + 1) * 512]
                         for kt in range(8)] for hl in range(2)]
                woutT = [[wout_all[:, (hl * NDT + j) * DM:(hl * NDT + j + 1) * DM]
                          for j in range(NDT)] for hl in range(2)]
                wxpT = [[wxp_all[:, (hl * NDT + j) * 96:(hl * NDT + j + 1) * 96]
                         for j in range(NDT)] for hl in range(2)]
                wdtT = [wdt_all[:, hl * DL:(hl + 1) * DL] for hl in range(2)]
                cdg = [[[cd_all[:, ((hl * NDT + j) * DCONV + k) * 128:
                                ((hl * NDT + j) * DCONV + k + 1) * 128]
                         for k in range(DCONV)] for j in range(NDT)] for hl in range(2)]
                ddg = [[dd_all[:, (hl * NDT + j) * 128:(hl * NDT + j + 1) * 128]
                        for j in range(NDT)] for hl in range(2)]
                convb, dtb, Asb = [], [], []
                for j in range(NDT):
                    cb = wp.tile([128, 1], F32, tag=f"convb{j}")
                    nc.sync.dma_start(cb[:], conv_b[l, j * 128:(j + 1) * 128, :])
                    convb.append(cb)
                    db = wp.tile([128, 1], F32, tag=f"dtb{j}")
                    nc.sync.dma_start(db[:], dt_b[l, j * 128:(j + 1) * 128, :])
                    dtb.append(db)
                    at = wp.tile([128, DS], F32, tag=f"alog{j}")
                    nc.sync.dma_start(at[:], a_log[l, j * 128:(j + 1) * 128, :])
                    ae = wp.tile([128, DS], F32, tag=f"aexp{j}")
                    nc.scalar.activation(ae[:], at[:], AF.Exp)
                    an = wp.tile([128, DS], F32, tag=f"aneg{j}")
                    nc.vector.tensor_scalar_mul(an[:], ae[:], -1.0)
                    Asb.append(an)
                if apply_norm_w:
                    nw_sb = wp.tile([128, DM], F32, tag="nwsb")
                    nc.sync.dma_start(nw_sb[:], nwb[l, :, :])
                if apply_norm_b:
                    nb_sb = wp.tile([128, DM], F32, tag="nbsb")
                    nc.sync.dma_start(nb_sb[:], nbb[l, :, :])

                # per-layer resident SBUF tensors
                u_sb = [wp.tile([128, T], BF16, tag=f"u{j}", name=f"u{l}_{j}")
                        for j in range(NDT)]
                z_sb = [wp.tile([128, T], BF16, tag=f"z{j}", name=f"z{l}_{j}")
                        for j in range(NDT)]

                # DRAM staging for collectives
                xdbl_in = [dram.tile([96, L], BF16, tag=f"xdbli{b}",
                                     name=f"xdbli{l}_{b}") for b in range(B)]
                xdbl_sh = [dram.tile([96, L], BF16, tag=f"xdblo{b}", addr_space="Shared",
                                     name=f"xdblo{l}_{b}") for b in range(B)]
                odt = BF16
                out_part = [[dram.tile([LH, DM], odt, tag=f"opart{l}_{b}_{h}",
                                       name=f"opart{l}_{b}_{h}") for h in range(2)]
                            for b in range(B)]
                hred = [[dram.tile([LH, DM], odt, tag=f"hred{l}_{b}_{h}",
                                   addr_space="Shared",
                                   name=f"hred{l}_{b}_{h}") for h in range(2)]
                        for b in range(B)]
                hred_ap = [[hred[b][h].opt() for h in range(2)] for b in range(B)]
                if l == DEPTH - 1:
                    final_hred = hred

                # ================= phase A: LN + transpose + in_proj + conv ===========
                prev_ue = [None, None]
                for ci in range(NCH):
                    b = ci // 4
                    tok0 = ci * 512
                    xa_t = [], 
                    xa_t = []
                    var4 = lnp.tile([128, 4], F32, tag="var4", bufs=2)
                    bna8 = lnp.tile([128, 8], F32, tag="bna8", bufs=2)
                    # stats on DVE: bn_stats (2x 512) + bn_aggr per t-tile
                    for tti in range(4):
                        row0 = (ci % 4) * 512 + tti * 128
                        xa = lnp.tile([128, DM], BF16, tag="xa", bufs=4)
                        nc.sync.dma_start(xa[:], hget(b, row0))
                        xa_t.append(xa)
                        bst = lnp.tile([128, 12], F32, tag="bst", bufs=2)
                        nc.vector.bn_stats(bst[:, 0:6], xa[:, 0:512])
                        nc.vector.bn_stats(bst[:, 6:12], xa[:, 512:1024])
                        nc.vector.bn_aggr(bna8[:, 2 * tti:2 * tti + 2], bst[:])
                        nc.vector.tensor_copy(var4[:, tti:tti + 1],
                                              bna8[:, 2 * tti + 1:2 * tti + 2])
                    # rstd: one Sqrt per chunk -- single table-swap cluster
                    std4 = lnp.tile([128, 4], F32, tag="std4", bufs=2)
                    nc.scalar.activation(std4[:], var4[:], AF.Sqrt, bias=eps_sb[:])
                    rstd4 = lnp.tile([128, 4], F32, tag="rstd4", bufs=2)
                    nc.vector.reciprocal(rstd4[:], std4[:])
                    hn_pack = lnp.tile([128, 4096], BF16, tag="hnpack", bufs=1)
                    for tti in range(4):
                        mean = bna8[:, 2 * tti:2 * tti + 1]
                        rstd = rstd4[:, tti:tti + 1]
                        nbias = lnp.tile([128, 1], F32, tag="nbias", bufs=4)
                        nc.vector.scalar_tensor_tensor(
                            nbias[:], mean, -1.0, rstd, ALU.mult, ALU.mult
                        )
                        hcol = hn_pack[:, tti * DM:(tti + 1) * DM]
                        if apply_norm_w or apply_norm_b:
                            hn0 = lnp.tile([128, DM], F32, tag="hn0", bufs=2)
                            nc.vector.tensor_scalar(
                                hn0[:], xa_t[tti][:], rstd, nbias[:],
                                ALU.mult, ALU.add,
                            )
                            if apply_norm_w and apply_norm_b:
                                hn1 = lnp.tile([128, DM], F32, tag="hn1", bufs=2)
                                nc.vector.tensor_mul(hn1[:], hn0[:], nw_sb[:])
                                nc.vector.tensor_add(hcol, hn1[:], nb_sb[:])
                            elif apply_norm_w:
                                nc.vector.tensor_mul(hcol, hn0[:], nw_sb[:])
                            else:
                                nc.vector.tensor_add(hcol, hn0[:], nb_sb[:])
                        else:
                            nc.vector.tensor_scalar(
                                hcol, xa_t[tti][:], rstd, nbias[:],
                                ALU.mult, ALU.add,
                            )
                    # transpose via DMA xbar: hnT[p, kt, t] = hn_pack[t', kt*128+p]
                    hnT = lnp.tile([128, 8, 512], BF16, tag="hnT", bufs=1)
                    for tti in range(4):
                        nc.sync.dma_start_transpose(
                            hnT[:, :, tti * 128:(tti + 1) * 128],
                            hn_pack[:, tti * DM:(tti + 1) * DM],
                        )
                    # in_proj + conv + silu + x_proj
                    for mt in range(4):
                        pm = psA.tile([128, 512], F32, tag="pm")
                        for kt in range(8):
                            nc.tensor.matmul(
                                pm[:],
                                winT[0][kt][:, mt * 128:(mt + 1) * 128],
                                hnT[:, kt, :],
                                start=(kt == 0),
                                stop=(kt == 7),
                            )
                        if mt < NDT:
                            j = mt
                            ue = sp.tile([128, 515], BF16, tag=f"ue{j}", bufs=2)
                            if ci % 4 == 0:
                                nc.vector.memset(ue[:, 0:3], 0.0)
                            else:
                                nc.vector.tensor_copy(
                                    ue[:, 0:3], prev_ue[j][:, 512:515]
                                )
                            nc.scalar.copy(ue[:, 3:515], pm[:])
                            prev_ue[j] = ue
                            pcv = psA.tile([128, 512], F32, tag="pm")
                            for k in range(DCONV):
                                nc.tensor.matmul(
                                    pcv[:], cdg[0][j][k][:], ue[:, k:k + 512],
                                    start=(k == 0), stop=(k == DCONV - 1),
                                )
                            nc.scalar.activation(
                                u_sb[j][:, tok0:tok0 + 512], pcv[:],
                                AF.Silu, bias=convb[j][:],
                            )
                        else:
                            j = mt - NDT
                            nc.scalar.activation(
                                z_sb[j][:, tok0:tok0 + 512], pm[:], AF.Silu
                            )
                    px = psA.tile([96, 512], F32, tag="pm")
                    for hl in range(2):
                        for j in range(NDT):
                            nc.tensor.matmul(
                                px[:], wxpT[hl][j][:], u_sb[j][:, tok0:tok0 + 512],
                                start=(hl == 0 and j == 0),
                                stop=(hl == 1 and j == NDT - 1),
                            )
                    xdc = sp.tile([96, 512], BF16, tag="xdc", bufs=2)
                    nc.scalar.copy(xdc[:], px[:])
                    ctok = (ci % 4) * 512
                    nc.sync.dma_start(xdbl_in[b][:, ctok:ctok + 512], xdc[:])

                    if ci == 3:
                        all_reduce(xdbl_in[0].opt(), xdbl_sh[0].opt())

                # ============= phase D: dt + scan; phase E: out_proj =============
                for b in range(B):
                    if b == 1:
                        all_reduce(xdbl_in[1].opt(), xdbl_sh[1].opt())
                    xrd = sp.tile([DTR, L], BF16, tag="xrd")
                    nc.sync.dma_start(xrd[:], xdbl_sh[b][0:DTR, :])
                    dts, dus = [], []
                    for j in range(NDT):
                        dt_j = dp.tile([128, L], BF16, tag=f"dt{j}", bufs=1,
                                       name=f"dt{l}_{b}_{j}")
                        evs = []
                        for q in range(4):
                            pdm = psS.tile([128, 512], F32, tag="ps")
                            for hl in range(2):
                                nc.tensor.matmul(
                                    pdm[:],
                                    wdtT[hl][:, j * 128:(j + 1) * 128],
                                    xrd[:, q * 512:(q + 1) * 512],
                                    start=(hl == 0), stop=(hl == 1),
                                )
                            ev = sp.tile([128, 512], F32, tag="ev", bufs=4)
                            nc.scalar.activation(ev[:], pdm[:], AF.Exp, bias=dtb[j][:])
                            evs.append(ev)
                        for q in range(4):
                            nc.scalar.activation(
                                dt_j[:, q * 512:(q + 1) * 512], evs[q][:],
                                AF.Ln, bias=one_sb[:],
                            )
                        du_j = dp.tile([128, L], BF16, tag=f"du{j}", bufs=1,
                                       name=f"du{l}_{b}_{j}")
                        nc.gpsimd.tensor_tensor(
                            du_j[:], dt_j[:], u_sb[j][:, b * L:(b + 1) * L], ALU.mult
                        )
                        dts.append(dt_j)
                        dus.append(du_j)
                    for j in range(NDT):
                        y_ps = psY.tile([128, L], F32, tag="yps")
                        for n in range(DS):
                            pb = bbp.tile([128, L], BF16, tag="pb")
                            nc.sync.dma_start(
                                pb[:],
                                xdbl_sh[b][DTR + n:DTR + n + 1, :].to_broadcast((128, L)),
                            )
                            pc = bcp.tile([128, L], BF16, tag="pc")
                            nc.sync.dma_start(
                                pc[:],
                                xdbl_sh[b][DTR + DS + n:DTR + DS + n + 1, :]
                                .to_broadcast((128, L)),
                            )
                            ada = dp.tile([128, L], BF16, tag="ada")
                            nc.scalar.activation(
                                ada[:], dts[j][:], AF.Exp, scale=Asb[j][:, n:n + 1]
                            )
                            bt = dp.tile([128, L], BF16, tag="bt")
                            if n in BT_POOL_N:
                                nc.gpsimd.tensor_tensor(bt[:], dus[j][:], pb[:], ALU.mult)
                            else:
                                nc.vector.tensor_mul(bt[:], dus[j][:], pb[:])
                            hs = dp.tile([128, L], BF16, tag="hs")
                            nc.vector.tensor_tensor_scan(
                                hs[:], ada[:], bt[:], 0.0, ALU.mult, ALU.add
                            )
                            if n in YT_POOL_N:
                                nc.gpsimd.tensor_tensor(hs[:], hs[:], pc[:], ALU.mult)
                            else:
                                nc.vector.tensor_mul(hs[:], hs[:], pc[:])
                            for q in range(4):
                                nc.tensor.matmul(
                                    y_ps[:, q * 512:(q + 1) * 512],
                                    ident_sb[:],
                                    hs[:, q * 512:(q + 1) * 512],
                                    start=(n == 0), stop=False,
                                )
                        # D*u skip term closes the accumulation
                        for hl in range(2):
                            for q in range(4):
                                nc.tensor.matmul(
                                    y_ps[:, q * 512:(q + 1) * 512],
                                    ddg[hl][j][:],
                                    u_sb[j][:, b * L + q * 512: b * L + (q + 1) * 512],
                                    start=False, stop=(hl == 1),
                                )
                        y2 = dp.tile([128, L], BF16, tag=f"y2{j}", bufs=1)
                        nc.vector.tensor_mul(
                            y2[:], y_ps[:], z_sb[j][:, b * L:(b + 1) * L]
                        )
                        dts[j] = None
                        if j == 0:
                            y2s = [y2]
                        else:
                            y2s.append(y2)
                    # phase E: out_proj in token halves, AllReduce each half
                    for h in range(2):
                        for tt in range(8):
                            t0 = h * LH + tt * 128
                            for nt in range(2):
                                po = psS.tile([128, 512], F32, tag="ps")
                                for hl in range(2):
                                    for j in range(NDT):
                                        nc.tensor.matmul(
                                            po[:],
                                            y2s[j][:, t0:t0 + 128],
                                            woutT[hl][j][:, nt * 512:(nt + 1) * 512],
                                            start=(hl == 0 and j == 0),
                                            stop=(hl == 1 and j == NDT - 1),
                                        )
                                oc = sp.tile([128, 512], odt, tag="oc", bufs=3)
                                nc.scalar.copy(oc[:], po[:])
                                nc.sync.dma_start(
                                    out_part[b][h][tt * 128:(tt + 1) * 128,
                                                   nt * 512:(nt + 1) * 512],
                                    oc[:],
                                )
                        all_reduce(out_part[b][h].opt(), hred_ap[b][h])

                def mk_hget(hred_l):
                    def _g(b, row0):
                        h = row0 // LH
                        r = row0 % LH
                        return hred_l[b][h][r:r + 128, :]
                    return _g

                hget = mk_hget(hred)

            # final: cast bf16 -> f32 and store
            for b in range(B):
                for h in range(2):
                    for rt in range(LH // 128):
                        ld = sp.tile([128, DM], BF16, tag="fld", bufs=2)
                        nc.sync.dma_start(
                            ld[:], final_hred[b][h][rt * 128:(rt + 1) * 128, :])
                        fc = sp.tile([128, DM], F32, tag="ffc", bufs=2)
                        if rt % 2 == 0:
                            nc.scalar.copy(fc[:], ld[:])
                        else:
                            nc.vector.tensor_copy(fc[:], ld[:])
                        nc.sync.dma_start(
                            out_dram[b * L + h * LH + rt * 128:
                                     b * L + h * LH + (rt + 1) * 128, :],
                            fc[:],
                        )

    nc.compile()
    return nc


_CACHE = {}


def _get_nc(apply_norm_w, apply_norm_b, fake_cc=False):
    key = (apply_norm_w, apply_norm_b, fake_cc)
    if key not in _CACHE:
        _CACHE[key] = build_nc(apply_norm_w, apply_norm_b, fake_cc)
    return _CACHE[key]


def make_in_maps(x, norm_w, norm_b, in_proj_w, conv_w, conv_b, x_proj_w,
                 dt_proj_w, dt_proj_b, A_log, D, out_proj_w,
                 apply_norm_w, apply_norm_b):
    bf = mybir.dt.np(BF16)
    f = lambda a: np.ascontiguousarray(np.asarray(a), dtype=np.float32)
    fb = lambda a: np.ascontiguousarray(np.asarray(a, dtype=np.float32).astype(bf))

    def hilo(a):
        a = np.asarray(a, dtype=np.float32)
        hi = a.astype(bf)
        lo = (a - hi.astype(np.float32)).astype(bf)
        return np.ascontiguousarray(np.stack([hi, lo], axis=0))

    def pack_in(a):  # [2, DEPTH, DM, 512] -> [DEPTH, 128, 2*8*512]
        a = a.reshape(2, DEPTH, 8, 128, 512)
        return np.ascontiguousarray(
            a.transpose(1, 3, 0, 2, 4).reshape(DEPTH, 128, 2 * 8 * 512))

    def pack_pj(a, w):  # [2, DEPTH, DL, w] -> [DEPTH, 128, 2*NDT*w]
        a = a.reshape(2, DEPTH, NDT, 128, w)
        return np.ascontiguousarray(
            a.transpose(1, 3, 0, 2, 4).reshape(DEPTH, 128, 2 * NDT * w))

    def pack_dt(a):  # [2, DEPTH, DTR, DL] -> [DEPTH, DTR, 2*DL]
        return np.ascontiguousarray(
            a.transpose(1, 2, 0, 3).reshape(DEPTH, DTR, 2 * DL))

    def pack_cd(a):  # [2, DEPTH, NDT, DCONV, 128, 128] -> [DEPTH, 128, 2*NDT*DCONV*128]
        return np.ascontiguousarray(
            a.transpose(1, 4, 0, 2, 3, 5).reshape(DEPTH, 128, 2 * NDT * DCONV * 128))

    def pack_dd(a):  # [2, DEPTH, NDT, 128, 128] -> [DEPTH, 128, 2*NDT*128]
        return np.ascontiguousarray(
            a.transpose(1, 3, 0, 2, 4).reshape(DEPTH, 128, 2 * NDT * 128))

    x_tm = fb(np.asarray(x).reshape(T, DM))
    in_proj_w = np.asarray(in_proj_w)
    conv_w = np.asarray(conv_w)
    D_np = np.asarray(D)
    in_maps = []
    for c in range(NCORES):
        sl = slice(c * DL, (c + 1) * DL)
        w_in_rows = np.concatenate(
            [in_proj_w[:, sl, :], in_proj_w[:, DI + c * DL: DI + (c + 1) * DL, :]],
            axis=1,
        )  # (DEPTH, 512, 1024)
        cdg = np.zeros((DEPTH, NDT, DCONV, 128, 128), dtype=np.float32)
        ddg = np.zeros((DEPTH, NDT, 128, 128), dtype=np.float32)
        for li in range(DEPTH):
            for j in range(NDT):
                ch = slice(c * DL + j * 128, c * DL + (j + 1) * 128)
                for k in range(DCONV):
                    np.fill_diagonal(cdg[li, j, k], conv_w[li, ch, 0, k])
                np.fill_diagonal(ddg[li, j], D_np[li, ch])
        m = {
            "x_tm": x_tm,
            "w_inT": pack_in(hilo(w_in_rows.transpose(0, 2, 1))),
            "w_outT": pack_pj(hilo(np.asarray(out_proj_w)[:, :, sl].transpose(0, 2, 1)), DM),
            "w_xpT": pack_pj(hilo(np.asarray(x_proj_w)[:, :, sl].transpose(0, 2, 1)), 96),
            "w_dtT": pack_dt(hilo(np.asarray(dt_proj_w)[:, sl, :].transpose(0, 2, 1))),
            "conv_dg": pack_cd(hilo(cdg)),
            "d_dg": pack_dd(hilo(ddg)),
            "conv_b_c": f(np.asarray(conv_b)[:, sl][..., None]),
            "dt_b_c": f(np.asarray(dt_proj_b)[:, sl][..., None]),
            "a_log_c": f(np.asarray(A_log)[:, sl, :]),
            "ident_bf": np.eye(128, dtype=np.float32).astype(bf),
        }
        if apply_norm_w:
            m["norm_w_bc"] = f(np.broadcast_to(np.asarray(norm_w)[:, None, :], (DEPTH, 128, DM)))
        if apply_norm_b:
            m["norm_b_bc"] = f(np.broadcast_to(np.asarray(norm_b)[:, None, :], (DEPTH, 128, DM)))
        in_maps.append(m)
    return in_maps


def kernel(x, x_size, norm_w, norm_b, in_proj_w, conv_w, conv_b, x_proj_w,
           dt_proj_w, dt_proj_b, A_log, D, out_proj_w, **_unused):
    apply_norm_w = not np.allclose(np.asarray(norm_w), 1.0)
    apply_norm_b = not np.allclose(np.asarray(norm_b), 0.0)
    nc = _get_nc(apply_norm_w, apply_norm_b)
    in_maps = make_in_maps(
        x, norm_w, norm_b, in_proj_w, conv_w, conv_b, x_proj_w,
        dt_proj_w, dt_proj_b, A_log, D, out_proj_w,
        apply_norm_w, apply_norm_b,
    )
    res = run_bass_kernel_spmd(nc, in_maps, core_ids=list(range(NCORES)))
    return res.results[0]["out_tm"].reshape(B, L, DM).astype(np.float32)


# revision 23
# speedup vs baseline: 2.0121x; 1.3399x over previous
"""Trainium2 Bass kernel for a 2-layer Mamba stack (BasicLayer). v2.

Per layer: LayerNorm -> in_proj (1024->4096) -> causal depthwise conv(k=4)
+ SiLU -> x_proj (2048->96) -> dt_proj + softplus -> selective scan over
L=2048 -> gate with SiLU(z) -> out_proj (2048->1024).

Sharding: tensor-parallel over d_inner (2048 / 8 cores = 256 channels per
core).  Cross-core sums (x_proj and out_proj contractions) are AllReduced
on-chip in bf16, out_proj split in token halves so the collectives overlap
compute.  All matmuls run in bf16 (fp32 matmul is 4 cycles/row vs 1 for
bf16).  Transposes use the DMA xbar (dma_start_transpose) instead of the
PE+PSUM path.  The depthwise conv and the D*u skip term are expressed as
diagonal-matrix matmuls on the PE so the vector engine only carries the
scan itself plus the B/C elementwise products.  softplus(x) is computed as
Ln(1+Exp(x)) -- both functions live in the same activation table, and the
LayerNorm rstd is the only per-chunk table swap (Sqrt).
"""

import numpy as np

try:
    import concourse.bass as bass
except ImportError:  # pragma: no cover
    import sys

    sys.path.insert(0, "/opt/trn_rl_repo")
    import concourse.bass as bass

import concourse.bacc as bacc
import concourse.mybir as mybir
import concourse.tile as tile
from concourse.bass_utils import run_bass_kernel_spmd

F32 = mybir.dt.float32
BF16 = mybir.dt.bfloat16
AF = mybir.ActivationFunctionType
ALU = mybir.AluOpType

B, L = 2, 2048
DM, DI, DS, DTR, DCONV, DEPTH = 1024, 2048, 16, 64, 4, 2
EPS = 1e-5
NCORES = 8
DL = DI // NCORES          # 256 channels per core
NDT = DL // 128            # 2 channel tiles per core
T = B * L                  # 4096 tokens
NCH = T // 512             # 8 chunks of 512 tokens
LH = L // 2                # token half for out AllReduce chunking

# knobs
YT_POOL_N = set()          # scan ns whose yt-mul runs on gpsimd instead of DVE
BT_POOL_N = {1, 3, 5, 7, 9, 11, 13, 15}


def build_nc(apply_norm_w: bool, apply_norm_b: bool, fake_cc: bool = False):
    nc = bacc.Bacc(
        "TRN2",
        target_bir_lowering=False,
        debug=False,
        enable_asserts=False,
        num_devices=NCORES,
    )

    # ---- I/O declarations (per-core data supplied via in_maps) ----
    x_dram = nc.dram_tensor("x_tm", [T, DM], BF16, kind="ExternalInput")
    # packed per-layer weights: big contiguous blocks to minimize DMA count
    w_inT = nc.dram_tensor("w_inT", [DEPTH, 128, 2 * 8 * 512], BF16, kind="ExternalInput")
    w_outT = nc.dram_tensor("w_outT", [DEPTH, 128, 2 * NDT * DM], BF16, kind="ExternalInput")
    w_xpT = nc.dram_tensor("w_xpT", [DEPTH, 128, 2 * NDT * 96], BF16, kind="ExternalInput")
    w_dtT = nc.dram_tensor("w_dtT", [DEPTH, DTR, 2 * DL], BF16, kind="ExternalInput")
    conv_dg = nc.dram_tensor("conv_dg", [DEPTH, 128, 2 * NDT * DCONV * 128], BF16,
                             kind="ExternalInput")
    d_dg = nc.dram_tensor("d_dg", [DEPTH, 128, 2 * NDT * 128], BF16, kind="ExternalInput")
    conv_b = nc.dram_tensor("conv_b_c", [DEPTH, DL, 1], F32, kind="ExternalInput")
    dt_b = nc.dram_tensor("dt_b_c", [DEPTH, DL, 1], F32, kind="ExternalInput")
    a_log = nc.dram_tensor("a_log_c", [DEPTH, DL, DS], F32, kind="ExternalInput")
    ident = nc.dram_tensor("ident_bf", [128, 128], BF16, kind="ExternalInput")
    if apply_norm_w:
        nwb = nc.dram_tensor("norm_w_bc", [DEPTH, 128, DM], F32, kind="ExternalInput")
    if apply_norm_b:
        nbb = nc.dram_tensor("norm_b_bc", [DEPTH, 128, DM], F32, kind="ExternalInput")
    out_dram = nc.dram_tensor("out_tm", [T, DM], F32, kind="ExternalOutput")

    groups = [list(range(NCORES))]

    def all_reduce(src_ap, dst_ap):
        if fake_cc:
            nc.sync.dma_start(dst_ap, src_ap)
        else:
            nc.gpsimd.collective_compute(
                "AllReduce", ALU.add, replica_groups=groups,
                ins=[src_ap], outs=[dst_ap],
            )

    with tile.TileContext(nc, num_cores=NCORES) as tc:
        with (
            tc.tile_pool(name="wp", bufs=1) as wp,
            tc.tile_pool(name="lnp", bufs=2) as lnp,
            tc.tile_pool(name="sp", bufs=2) as sp,
            tc.tile_pool(name="dp", bufs=2) as dp,
            tc.tile_pool(name="bbp", bufs=3) as bbp,
            tc.tile_pool(name="bcp", bufs=3) as bcp,
            tc.tile_pool(name="psA", bufs=2, space="PSUM") as psA,
            tc.tile_pool(name="psY", bufs=1, space="PSUM") as psY,
            tc.tile_pool(name="psS", bufs=2, space="PSUM") as psS,
            tc.tile_pool(name="dram", bufs=2, space="DRAM") as dram,
        ):
            ident_sb = wp.tile([128, 128], BF16, tag="ident")
            nc.sync.dma_start(ident_sb[:], ident[:, :])
            eps_sb = wp.tile([128, 1], F32, tag="eps")
            nc.vector.memset(eps_sb[:], EPS)
            one_sb = wp.tile([128, 1], F32, tag="one")
            nc.vector.memset(one_sb[:], 1.0)

            # hsrc(b, row0) -> AP of 128 input rows for this layer
            hsrc_l0 = [x_dram.ap()[0:L, :], x_dram.ap()[L:T, :]]

            def hsrc_l0_get(b, row0):
                return hsrc_l0[b][row0:row0 + 128, :]

            hget = hsrc_l0_get

            for l in range(DEPTH):
                # ---- per-layer weights ----
                win_all = wp.tile([128, 2 * 8 * 512], BF16, tag="winall")
                nc.sync.dma_start(win_all[:], w_inT[l, :, :])
                wout_all = wp.tile([128, 2 * NDT * DM], BF16, tag="woutall")
                nc.sync.dma_start(wout_all[:], w_outT[l, :, :])
                wxp_all = wp.tile([128, 2 * NDT * 96], BF16, tag="wxpall")
                nc.sync.dma_start(wxp_all[:], w_xpT[l, :, :])
                wdt_all = wp.tile([DTR, 2 * DL], BF16, tag="wdtall")
                nc.sync.dma_start(wdt_all[:], w_dtT[l, :, :])
                cd_all = wp.tile([128, 2 * NDT * DCONV * 128], BF16, tag="cdall")
                nc.sync.dma_start(cd_all[:], conv_dg[l, :, :])
                dd_all = wp.tile([128, 2 * NDT * 128], BF16, tag="ddall")
                nc.sync.dma_start(dd_all[:], d_dg[l, :, :])
                winT = [[win_all[:, (hl * 8 + kt) * 512:(hl * 8 + kt + 1) * 512]
                         for kt in range(8)] for hl in range(2)]
                woutT = [[wout_all[:, (hl * NDT + j) * DM:(hl * NDT + j + 1) * DM]
                          for j in range(NDT)] for hl in range(2)]
                wxpT = [[wxp_all[:, (hl * NDT + j) * 96:(hl * NDT + j + 1) * 96]
                         for j in range(NDT)] for hl in range(2)]
                wdtT = [wdt_all[:, hl * DL:(hl + 1) * DL] for hl in range(2)]
                cdg = [[[cd_all[:, ((hl * NDT + j) * DCONV + k) * 128:
                                ((hl * NDT + j) * DCONV + k + 1) * 128]
                         for k in range(DCONV)] for j in range(NDT)] for hl in range(2)]
                ddg = [[dd_all[:, (hl * NDT + j) * 128:(hl * NDT + j + 1) * 128]
                        for j in range(NDT)] for hl in range(2)]
                convb, dtb, Asb = [], [], []
                for j in range(NDT):
                    cb = wp.tile([128, 1], F32, tag=f"convb{j}")
                    nc.sync.dma_start(cb[:], conv_b[l, j * 128:(j + 1) * 128, :])
                    convb.append(cb)
                    db = wp.tile([128, 1], F32, tag=f"dtb{j}")
                    nc.sync.dma_start(db[:], dt_b[l, j * 128:(j + 1) * 128, :])
                    dtb.append(db)
                    at = wp.tile([128, DS], F32, tag=f"alog{j}")
                    nc.sync.dma_start(at[:], a_log[l, j * 128:(j + 1) * 128, :])
                    ae = wp.tile([128, DS], F32, tag=f"aexp{j}")
                    nc.scalar.activation(ae[:], at[:], AF.Exp)
                    an = wp.tile([128, DS], F32, tag=f"aneg{j}")
                    nc.vector.tensor_scalar_mul(an[:], ae[:], -1.0)
                    Asb.append(an)
                if apply_norm_w:
                    nw_sb = wp.tile([128, DM], F32, tag="nwsb")
                    nc.sync.dma_start(nw_sb[:], nwb[l, :, :])
                if apply_norm_b:
                    nb_sb = wp.tile([128, DM], F32, tag="nbsb")
                    nc.sync.dma_start(nb_sb[:], nbb[l, :, :])

                # per-layer resident SBUF tensors
                u_sb = [wp.tile([128, T], BF16, tag=f"u{j}", name=f"u{l}_{j}")
                        for j in range(NDT)]
                z_sb = [wp.tile([128, T], BF16, tag=f"z{j}", name=f"z{l}_{j}")
                        for j in range(NDT)]

                # DRAM staging for collectives
                xdbl_in = [dram.tile([96, L], BF16, tag=f"xdbli{b}",
                                     name=f"xdbli{l}_{b}") for b in range(B)]
                xdbl_sh = [dram.tile([96, L], BF16, tag=f"xdblo{b}", addr_space="Shared",
                                     name=f"xdblo{l}_{b}") for b in range(B)]
                odt = BF16
                out_part = [[dram.tile([LH, DM], odt, tag=f"opart{l}_{b}_{h}",
                                       name=f"opart{l}_{b}_{h}") for h in range(2)]
                            for b in range(B)]
                hred = [[dram.tile([LH, DM], odt, tag=f"hred{l}_{b}_{h}",
                                   addr_space="Shared",
                                   name=f"hred{l}_{b}_{h}") for h in range(2)]
                        for b in range(B)]
                hred_ap = [[hred[b][h].opt() for h in range(2)] for b in range(B)]
                if l == DEPTH - 1:
                    final_hred = hred

                # ================= phase A: LN + transpose + in_proj + conv ===========
                prev_ue = [None, None]
                for ci in range(NCH):
                    b = ci // 4
                    tok0 = ci * 512
                    xa_t = [], 
                    xa_t = []
                    var4 = lnp.tile([128, 4], F32, tag="var4", bufs=2)
                    bna8 = lnp.tile([128, 8], F32, tag="bna8", bufs=2)
                    # stats on DVE: bn_stats (2x 512) + bn_aggr per t-tile
                    for tti in range(4):
                        row0 = (ci % 4) * 512 + tti * 128
                        xa = lnp.tile([128, DM], BF16, tag="xa", bufs=4)
                        nc.sync.dma_start(xa[:], hget(b, row0))
                        xa_t.append(xa)
                        bst = lnp.tile([128, 12], F32, tag="bst", bufs=2)
                        nc.vector.bn_stats(bst[:, 0:6], xa[:, 0:512])
                        nc.vector.bn_stats(bst[:, 6:12], xa[:, 512:1024])
                        nc.vector.bn_aggr(bna8[:, 2 * tti:2 * tti + 2], bst[:])
                        nc.vector.tensor_copy(var4[:, tti:tti + 1],
                                              bna8[:, 2 * tti + 1:2 * tti + 2])
                    # rstd: one Sqrt per chunk -- single table-swap cluster
                    std4 = lnp.tile([128, 4], F32, tag="std4", bufs=2)
                    nc.scalar.activation(std4[:], var4[:], AF.Sqrt, bias=eps_sb[:])
                    rstd4 = lnp.tile([128, 4], F32, tag="rstd4", bufs=2)
                    nc.vector.reciprocal(rstd4[:], std4[:])
                    hn_pack = lnp.tile([128, 4096], BF16, tag="hnpack", bufs=1)
                    for tti in range(4):
                        mean = bna8[:, 2 * tti:2 * tti + 1]
                        rstd = rstd4[:, tti:tti + 1]
                        nbias = lnp.tile([128, 1], F32, tag="nbias", bufs=4)
                        nc.vector.scalar_tensor_tensor(
                            nbias[:], mean, -1.0, rstd, ALU.mult, ALU.mult
                        )
                        hcol = hn_pack[:, tti * DM:(tti + 1) * DM]
                        if apply_norm_w or apply_norm_b:
                            hn0 = lnp.tile([128, DM], F32, tag="hn0", bufs=2)
                            nc.vector.tensor_scalar(
                                hn0[:], xa_t[tti][:], rstd, nbias[:],
                                ALU.mult, ALU.add,
                            )
                            if apply_norm_w and apply_norm_b:
                                hn1 = lnp.tile([128, DM], F32, tag="hn1", bufs=2)
                                nc.vector.tensor_mul(hn1[:], hn0[:], nw_sb[:])
                                nc.vector.tensor_add(hcol, hn1[:], nb_sb[:])
                            elif apply_norm_w:
                                nc.vector.tensor_mul(hcol, hn0[:], nw_sb[:])
                            else:
                                nc.vector.tensor_add(hcol, hn0[:], nb_sb[:])
                        else:
                            nc.vector.tensor_scalar(
                                hcol, xa_t[tti][:], rstd, nbias[:],
                                ALU.mult, ALU.add,
                            )
                    # transpose via DMA xbar: hnT[p, kt, t] = hn_pack[t', kt*128+p]
                    hnT = lnp.tile([128, 8, 512], BF16, tag="hnT", bufs=1)
                    for tti in range(4):
                        nc.sync.dma_start_transpose(
                            hnT[:, :, tti * 128:(tti + 1) * 128],
                            hn_pack[:, tti * DM:(tti + 1) * DM],
                        )
                    # in_proj + conv + silu + x_proj
                    for mt in range(4):
                        pm = psA.tile([128, 512], F32, tag="pm")
                        for kt in range(8):
                            nc.tensor.matmul(
                                pm[:],
                                winT[0][kt][:, mt * 128:(mt + 1) * 128],
                                hnT[:, kt, :],
                                start=(kt == 0),
                                stop=(kt == 7),
                            )
                        if mt < NDT:
                            j = mt
                            ue = sp.tile([128, 515], BF16, tag=f"ue{j}", bufs=2)
                            if ci % 4 == 0:
                                nc.vector.memset(ue[:, 0:3], 0.0)
                            else:
                                nc.vector.tensor_copy(
                                    ue[:, 0:3], prev_ue[j][:, 512:515]
                                )
                            nc.scalar.copy(ue[:, 3:515], pm[:])
                            prev_ue[j] = ue
                            pcv = psA.tile([128, 512], F32, tag="pm")
                            for k in range(DCONV):
                                nc.tensor.matmul(
                                    pcv[:], cdg[0][j][k][:], ue[:, k:k + 512],
                                    start=(k == 0), stop=(k == DCONV - 1),
                                )
                            nc.scalar.activation(
                                u_sb[j][:, tok0:tok0 + 512], pcv[:],
                                AF.Silu, bias=convb[j][:],
                            )
                        else:
                            j = mt - NDT
                            nc.scalar.activation(
                                z_sb[j][:, tok0:tok0 + 512], pm[:], AF.Silu
                            )
                    px = psA.tile([96, 512], F32, tag="pm")
                    for hl in range(2):
                        for j in range(NDT):
                            nc.tensor.matmul(
                                px[:], wxpT[hl][j][:], u_sb[j][:, tok0:tok0 + 512],
                                start=(hl == 0 and j == 0),
                                stop=(hl == 1 and j == NDT - 1),
                            )
                    xdc = sp.tile([96, 512], BF16, tag="xdc", bufs=2)
                    nc.scalar.copy(xdc[:], px[:])
                    ctok = (ci % 4) * 512
                    nc.sync.dma_start(xdbl_in[b][:, ctok:ctok + 512], xdc[:])

                    if ci == 3:
                        all_reduce(xdbl_in[0].opt(), xdbl_sh[0].opt())

                # ============= phase D: dt + scan; phase E: out_proj =============
                for b in range(B):
                    if b == 1:
                        all_reduce(xdbl_in[1].opt(), xdbl_sh[1].opt())
                    xrd = sp.tile([DTR, L], BF16, tag="xrd")
                    nc.sync.dma_start(xrd[:], xdbl_sh[b][0:DTR, :])
                    dts, dus = [], []
                    for j in range(NDT):
                        dt_j = dp.tile([128, L], BF16, tag=f"dt{j}", bufs=1,
                                       name=f"dt{l}_{b}_{j}")
                        evs = []
                        for q in range(4):
                            pdm = psS.tile([128, 512], F32, tag="ps")
                            for hl in range(2):
                                nc.tensor.matmul(
                                    pdm[:],
                                    wdtT[hl][:, j * 128:(j + 1) * 128],
                                    xrd[:, q * 512:(q + 1) * 512],
                                    start=(hl == 0), stop=(hl == 1),
                                )
                            ev = sp.tile([128, 512], F32, tag="ev", bufs=4)
                            nc.scalar.activation(ev[:], pdm[:], AF.Exp, bias=dtb[j][:])
                            evs.append(ev)
                        for q in range(4):
                            nc.scalar.activation(
                                dt_j[:, q * 512:(q + 1) * 512], evs[q][:],
                                AF.Ln, bias=one_sb[:],
                            )
                        du_j = dp.tile([128, L], BF16, tag=f"du{j}", bufs=1,
                                       name=f"du{l}_{b}_{j}")
                        nc.gpsimd.tensor_tensor(
                            du_j[:], dt_j[:], u_sb[j][:, b * L:(b + 1) * L], ALU.mult
                        )
                        dts.append(dt_j)
                        dus.append(du_j)
                    for j in range(NDT):
                        y_ps = psY.tile([128, L], F32, tag="yps")
                        for n in range(DS):
                            pb = bbp.tile([128, L], BF16, tag="pb")
                            nc.scalar.dma_start(
                                pb[:],
                                xdbl_sh[b][DTR + n:DTR + n + 1, :].to_broadcast((128, L)),
                            )
                            pc = bcp.tile([128, L], BF16, tag="pc")
                            nc.scalar.dma_start(
                                pc[:],
                                xdbl_sh[b][DTR + DS + n:DTR + DS + n + 1, :]
                                .to_broadcast((128, L)),
                            )
                            ada = dp.tile([128, L], BF16, tag="ada")
                            nc.scalar.activation(
                                ada[:], dts[j][:], AF.Exp, scale=Asb[j][:, n:n + 1]
                            )
                            bt = dp.tile([128, L], BF16, tag="bt")
                            if n in BT_POOL_N:
                                nc.gpsimd.tensor_tensor(bt[:], dus[j][:], pb[:], ALU.mult)
                            else:
                                nc.vector.tensor_mul(bt[:], dus[j][:], pb[:])
                            hs = dp.tile([128, L], BF16, tag="hs")
                            nc.vector.tensor_tensor_scan(
                                hs[:], ada[:], bt[:], 0.0, ALU.mult, ALU.add
                            )
                            yt = dp.tile([128, L], BF16, tag="yt")
                            if n in YT_POOL_N:
                                nc.gpsimd.tensor_tensor(yt[:], hs[:], pc[:], ALU.mult)
                            else:
                                nc.vector.tensor_mul(yt[:], hs[:], pc[:])
                            for q in range(4):
                                nc.tensor.matmul(
                                    y_ps[:, q * 512:(q + 1) * 512],
                                    ident_sb[:],
                                    yt[:, q * 512:(q + 1) * 512],
                                    start=(n == 0), stop=False,
                                )
                        # D*u skip term closes the accumulation
                        for hl in range(2):
                            for q in range(4):
                                nc.tensor.matmul(
                                    y_ps[:, q * 512:(q + 1) * 512],
                                    ddg[hl][j][:],
                                    u_sb[j][:, b * L + q * 512: b * L + (q + 1) * 512],
                                    start=False, stop=(hl == 1),
                                )
                        y2 = dp.tile([128, L], BF16, tag=f"y2{j}", bufs=1)
                        nc.vector.tensor_mul(
                            y2[:], y_ps[:], z_sb[j][:, b * L:(b + 1) * L]
                        )
                        dts[j] = None
                        if j == 0:
                            y2s = [y2]
                        else:
                            y2s.append(y2)
                    # phase E: out_proj in token halves, AllReduce each half
                    for h in range(2):
                        for tt in range(8):
                            t0 = h * LH + tt * 128
                            for nt in range(2):
                                po = psS.tile([128, 512], F32, tag="ps")
                                for hl in range(2):
                                    for j in range(NDT):
                                        nc.tensor.matmul(
                                            po[:],
                                            y2s[j][:, t0:t0 + 128],
                                            woutT[hl][j][:, nt * 512:(nt + 1) * 512],
                                            start=(hl == 0 and j == 0),
                                            stop=(hl == 1 and j == NDT - 1),
                                        )
                                oc = sp.tile([128, 512], odt, tag="oc", bufs=3)
                                nc.scalar.copy(oc[:], po[:])
                                nc.sync.dma_start(
                                    out_part[b][h][tt * 128:(tt + 1) * 128,
                                                   nt * 512:(nt + 1) * 512],
                                    oc[:],
                                )
                        all_reduce(out_part[b][h].opt(), hred_ap[b][h])

                def mk_hget(hred_l):
                    def _g(b, row0):
                        h = row0 // LH
                        r = row0 % LH
                        return hred_l[b][h][r:r + 128, :]
                    return _g

                hget = mk_hget(hred)

            # final: cast bf16 -> f32 and store
            for b in range(B):
                for h in range(2):
                    for rt in range(LH // 128):
                        ld = sp.tile([128, DM], BF16, tag="fld", bufs=2)
                        nc.sync.dma_start(
                            ld[:], final_hred[b][h][rt * 128:(rt + 1) * 128, :])
                        fc = sp.tile([128, DM], F32, tag="ffc", bufs=2)
                        if rt % 2 == 0:
                            nc.scalar.copy(fc[:], ld[:])
                        else:
                            nc.vector.tensor_copy(fc[:], ld[:])
                        nc.sync.dma_start(
                            out_dram[b * L + h * LH + rt * 128:
                                     b * L + h * LH + (rt + 1) * 128, :],
                            fc[:],
                        )

    nc.compile()
    return nc


_CACHE = {}


def _get_nc(apply_norm_w, apply_norm_b, fake_cc=False):
    key = (apply_norm_w, apply_norm_b, fake_cc)
    if key not in _CACHE:
        _CACHE[key] = build_nc(apply_norm_w, apply_norm_b, fake_cc)
    return _CACHE[key]


def make_in_maps(x, norm_w, norm_b, in_proj_w, conv_w, conv_b, x_proj_w,
                 dt_proj_w, dt_proj_b, A_log, D, out_proj_w,
                 apply_norm_w, apply_norm_b):
    bf = mybir.dt.np(BF16)
    f = lambda a: np.ascontiguousarray(np.asarray(a), dtype=np.float32)
    fb = lambda a: np.ascontiguousarray(np.asarray(a, dtype=np.float32).astype(bf))

    def hilo(a):
        a = np.asarray(a, dtype=np.float32)
        hi = a.astype(bf)
        lo = (a - hi.astype(np.float32)).astype(bf)
        return np.ascontiguousarray(np.stack([hi, lo], axis=0))

    def pack_in(a):  # [2, DEPTH, DM, 512] -> [DEPTH, 128, 2*8*512]
        a = a.reshape(2, DEPTH, 8, 128, 512)
        return np.ascontiguousarray(
            a.transpose(1, 3, 0, 2, 4).reshape(DEPTH, 128, 2 * 8 * 512))

    def pack_pj(a, w):  # [2, DEPTH, DL, w] -> [DEPTH, 128, 2*NDT*w]
        a = a.reshape(2, DEPTH, NDT, 128, w)
        return np.ascontiguousarray(
            a.transpose(1, 3, 0, 2, 4).reshape(DEPTH, 128, 2 * NDT * w))

    def pack_dt(a):  # [2, DEPTH, DTR, DL] -> [DEPTH, DTR, 2*DL]
        return np.ascontiguousarray(
            a.transpose(1, 2, 0, 3).reshape(DEPTH, DTR, 2 * DL))

    def pack_cd(a):  # [2, DEPTH, NDT, DCONV, 128, 128] -> [DEPTH, 128, 2*NDT*DCONV*128]
        return np.ascontiguousarray(
            a.transpose(1, 4, 0, 2, 3, 5).reshape(DEPTH, 128, 2 * NDT * DCONV * 128))

    def pack_dd(a):  # [2, DEPTH, NDT, 128, 128] -> [DEPTH, 128, 2*NDT*128]
        return np.ascontiguousarray(
            a.transpose(1, 3, 0, 2, 4).reshape(DEPTH, 128, 2 * NDT * 128))

    x_tm = fb(np.asarray(x).reshape(T, DM))
    in_proj_w = np.asarray(in_proj_w)
    conv_w = np.asarray(conv_w)
    D_np = np.asarray(D)
    in_maps = []
    for c in range(NCORES):
        sl = slice(c * DL, (c + 1) * DL)
        w_in_rows = np.concatenate(
            [in_proj_w[:, sl, :], in_proj_w[:, DI + c * DL: DI + (c + 1) * DL, :]],
            axis=1,
        )  # (DEPTH, 512, 1024)
        cdg = np.zeros((DEPTH, NDT, DCONV, 128, 128), dtype=np.float32)
        ddg = np.zeros((DEPTH, NDT, 128, 128), dtype=np.float32)
        for li in range(DEPTH):
            for j in range(NDT):
                ch = slice(c * DL + j * 128, c * DL + (j + 1) * 128)
                for k in range(DCONV):
                    np.fill_diagonal(cdg[li, j, k], conv_w[li, ch, 0, k])
                np.fill_diagonal(ddg[li, j], D_np[li, ch])
        m = {
            "x_tm": x_tm,
            "w_inT": pack_in(hilo(w_in_rows.transpose(0, 2, 1))),
            "w_outT": pack_pj(hilo(np.asarray(out_proj_w)[:, :, sl].transpose(0, 2, 1)), DM),
            "w_xpT": pack_pj(hilo(np.asarray(x_proj_w)[:, :, sl].transpose(0, 2, 1)), 96),
            "w_dtT": pack_dt(hilo(np.asarray(dt_proj_w)[:, sl, :].transpose(0, 2, 1))),
            "conv_dg": pack_cd(hilo(cdg)),
            "d_dg": pack_dd(hilo(ddg)),
            "conv_b_c": f(np.asarray(conv_b)[:, sl][..., None]),
            "dt_b_c": f(np.asarray(dt_proj_b)[:, sl][..., None]),
            "a_log_c": f(np.asarray(A_log)[:, sl, :]),
            "ident_bf": np.eye(128, dtype=np.float32).astype(bf),
        }
        if apply_norm_w:
            m["norm_w_bc"] = f(np.broadcast_to(np.asarray(norm_w)[:, None, :], (DEPTH, 128, DM)))
        if apply_norm_b:
            m["norm_b_bc"] = f(np.broadcast_to(np.asarray(norm_b)[:, None, :], (DEPTH, 128, DM)))
        in_maps.append(m)
    return in_maps


def kernel(x, x_size, norm_w, norm_b, in_proj_w, conv_w, conv_b, x_proj_w,
           dt_proj_w, dt_proj_b, A_log, D, out_proj_w, **_unused):
    apply_norm_w = not np.allclose(np.asarray(norm_w), 1.0)
    apply_norm_b = not np.allclose(np.asarray(norm_b), 0.0)
    nc = _get_nc(apply_norm_w, apply_norm_b)
    in_maps = make_in_maps(
        x, norm_w, norm_b, in_proj_w, conv_w, conv_b, x_proj_w,
        dt_proj_w, dt_proj_b, A_log, D, out_proj_w,
        apply_norm_w, apply_norm_b,
    )
    res = run_bass_kernel_spmd(nc, in_maps, core_ids=list(range(NCORES)))
    return res.results[0]["out_tm"].reshape(B, L, DM).astype(np.float32)
